# revision 50
# baseline (speedup 1.0000x reference)
"""Trainium2 Bass kernel for nn_Attention (LN -> QKV -> alibi attention -> out-proj).

Full shapes: x[2,2048,1024], alibi[1,16,2048,2048], w_qkv[1024,3072], w_out[1024,1024].
Sharding: tensor-parallel over heads. Core c owns heads {2c, 2c+1} for BOTH batches.
Each core computes a partial out-projection; the host sums the 8 partials (the
tensor-parallel reduction) and transposes back.

v2 design (all matmuls bf16 -- fp32r streams at 2 cyc/row on silicon, bf16 at 1):
  - x passed host-transposed + bf16: xT[b] = [d=1024, i=2048].
  - LN folded into the QKV eviction: qkv = rstd*(W^T x) + (mean*rstd)*(-colsum(W))
    (+ beta@W). LN stats (sum, sum-sq) via matmul-with-ones run concurrently with
    the QKV matmuls on raw x; no xn materialization, no LN->QKV serialization.
  - q/k evicted bf16 (2 heads on partitions); v bf16, PE-transposed to v-natural
    [j, 64d + ones-col]; the ones column makes attn@v also emit softmax denoms.
  - attention loops h outer, batch inner: each alibi^T tile (bf16, host-transposed)
    is DMA'd once and used by both batches (16MB/core alibi traffic, the minimum).
  - scores S^T = kT_chunk^T @ qT (K=64) + identity-matmul alibi accumulate; exp on
    ScalarE (PSUM f32 -> SBUF bf16), no max-subtraction (|scores| <~ 15).
  - PSUM: one shared [128,1024] pool (bufs=2) for stats/qkv/transpose/scores/
    out-proj + one [65,1024] pool (bufs=2) holding both batches' attn accumulators.
  - out partials written bf16 transposed [b, e, i]; host sums in f32.
"""

import sys

sys.path.insert(0, "/opt/trn_rl_repo")

from contextlib import ExitStack

import numpy as np
import ml_dtypes

import concourse.bass as bass
from concourse import bacc
import concourse.mybir as mybir
import concourse.tile as tile
from concourse.bass_utils import run_bass_kernel_spmd
from concourse.masks import make_identity

F32 = mybir.dt.float32
BF16 = mybir.dt.bfloat16

B, N, D = 2, 2048, 1024
H, DH = 16, 64
NCORES = 8
HL = H // NCORES          # local heads per core = 2
CL = HL * DH              # local head channels = 128
LN_EPS = 1e-5
SCALE = DH ** -0.5
KT = D // 128             # 8 d-tiles
JC = N // 128             # 16 j-chunks
IT = N // 512             # 4 i-tiles of 512

_CACHED_NC = None


def build_nc() -> bass.Bass:
    nc = bacc.Bacc(None)
    xt_d = nc.declare_dram_parameter("xt", [B, D, N], BF16, isOutput=False)
    al_d = nc.declare_dram_parameter("alibi", [HL, N, N], BF16, isOutput=False)
    wqkv_d = nc.declare_dram_parameter("wqkv", [D, 3 * CL], BF16, isOutput=False)
    qkvb_d = nc.declare_dram_parameter("qkvb", [3 * CL], F32, isOutput=False)
    nwsum_d = nc.declare_dram_parameter("nwsum", [3 * CL], F32, isOutput=False)
    wout_d = nc.declare_dram_parameter("wout", [CL, D], BF16, isOutput=False)
    ones_d = nc.declare_dram_parameter("ones", [128, 1], BF16, isOutput=False)
    out_d = nc.declare_dram_parameter("out", [B, D, N], BF16, isOutput=True)

    with tile.TileContext(nc) as tc, ExitStack() as ctx:
        ep = lambda **kw: ctx.enter_context(tc.tile_pool(**kw))
        cpool = ep(name="const", bufs=1)
        xt_pool = ep(name="xt", bufs=11)
        sq_pool = ep(name="sq", bufs=4)
        sm_pool = ep(name="small", bufs=2)
        tmp_pool = ep(name="tmp", bufs=4)
        qk_pool = ep(name="qk", bufs=1)      # per-batch tiles, all resident
        vt_pool = ep(name="vt", bufs=2)
        vn_pool = ep(name="vn", bufs=1)      # 4 resident tiles (b x head)
        al_pool = ep(name="al", bufs=8)
        at_pool = ep(name="at", bufs=8)
        ao_pool = ep(name="aos", bufs=1)
        ob_pool = ep(name="ob", bufs=4)
        bc_pool = ep(name="bc", bufs=4)
        rrbc_pool = ep(name="rrbc", bufs=3)
        aor_pool = ep(name="aor", bufs=3)
        dscr_pool = ep(name="dscr", bufs=2, space="DRAM")
        big_psum = ep(name="ps_big", bufs=3, space="PSUM")
        ao_psum = ep(name="ps_ao", bufs=1, space="PSUM")

        # ---- constants ----
        ident = cpool.tile([128, 128], BF16, name="ident")
        make_identity(nc, ident)
        zero_sb = cpool.tile([128, 1], F32, name="zero_sb")
        nc.vector.memset(zero_sb, 0.0)
        nc.const_aps.aps[(F32, 0.0)] = zero_sb[:, 0:1]
        eps_sb = cpool.tile([128, 1], F32, name="eps_sb")
        nc.vector.memset(eps_sb, LN_EPS)
        ones_sb = cpool.tile([128, 1], BF16, name="ones_sb")
        nc.sync.dma_start(out=ones_sb, in_=ones_d[:, :])
        wqkv_sb = cpool.tile([128, KT, 3 * CL], BF16, name="wqkv_sb")
        nc.sync.dma_start(out=wqkv_sb, in_=wqkv_d.rearrange("(t p) c -> p t c", p=128))
        qkvb_sb = cpool.tile([128, 3], F32, name="qkvb_sb")
        nc.sync.dma_start(out=qkvb_sb, in_=qkvb_d.rearrange("(c p) -> p c", p=128))
        nwsum_sb = cpool.tile([128, 3], F32, name="nwsum_sb")
        nc.sync.dma_start(out=nwsum_sb, in_=nwsum_d.rearrange("(c p) -> p c", p=128))
        wout_sb = cpool.tile([128, D], BF16, name="wout_sb")
        nc.sync.dma_start(out=wout_sb, in_=wout_d[:, :])

        qTs, kTs, vns, aos = [], [], [], []
        for b in range(B):
            # ---- load xT (bf16) ----
            xts = []
            for kt in range(KT):
                xt_t = xt_pool.tile([128, N], BF16, name=f"xt_{b}_{kt}", tag="xt")
                nc.sync.dma_start(out=xt_t, in_=xt_d[b, kt * 128:(kt + 1) * 128, :])
                xts.append(xt_t)

            # ---- LN stats (sum, sumsq) via matmul-with-ones ----

            scr = dscr_pool.tile([2, N], F32, name=f"scr_{b}", tag="scr")
            st = sm_pool.tile([128, 112], F32, name=f"st_{b}", tag="st128")
            for ihalf in range(2):
                isl = slice(ihalf * 1024, (ihalf + 1) * 1024)
                rows = sm_pool.tile([1, N], F32, name=f"rows_{b}_{ihalf}", tag="rows", bufs=1)
                sum_ps = big_psum.tile([1, 1024], F32, name=f"sum_{b}_{ihalf}", tag="big")
                sq_ps = big_psum.tile([33, 1024], F32, name=f"ssq_{b}_{ihalf}", tag="big")
                for kp in range(KT // 2):
                    kt = 2 * kp
                    # pairwise-reduce two d-tiles (bf16) before the matmul-ones
                    xp = sq_pool.tile([128, 1024], BF16, name=f"xp_{b}_{ihalf}_{kp}", tag="sq")
                    nc.gpsimd.tensor_add(xp, xts[kt][:, isl], xts[kt + 1][:, isl])
                    xsq = sq_pool.tile([128, 1024], BF16, name=f"xsq_{b}_{ihalf}_{kp}", tag="sq")
                    nc.gpsimd.tensor_mul(xsq, xts[kt][:, isl], xts[kt][:, isl])
                    xsq2 = sq_pool.tile([128, 1024], BF16, name=f"xsq2_{b}_{ihalf}_{kp}", tag="sq")
                    nc.gpsimd.tensor_mul(xsq2, xts[kt + 1][:, isl], xts[kt + 1][:, isl])
                    nc.vector.tensor_add(xsq, xsq, xsq2)
                    for it2 in range(2):
                        s2 = slice(it2 * 512, (it2 + 1) * 512)
                        nc.tensor.matmul(
                            sum_ps[0:1, s2], ones_sb, xp[:, s2],
                            start=(kp == 0), stop=(kp == KT // 2 - 1),
                        )
                        nc.tensor.matmul(
                            sq_ps[32:33, s2], ones_sb, xsq[:, s2],
                            start=(kp == 0), stop=(kp == KT // 2 - 1),
                            tile_position=(0, 32),
                        )
                nc.vector.tensor_copy(rows[0:1, 0:1024], sum_ps)
                nc.vector.tensor_copy(rows[0:1, 1024:2048], sq_ps[32:33, :])
                nc.sync.dma_start(out=st[:, ihalf * 8:(ihalf + 1) * 8], in_=rows[0:1, 0:1024])
                nc.sync.dma_start(out=st[:, 16 + ihalf * 8:16 + (ihalf + 1) * 8], in_=rows[0:1, 1024:2048])
            # stat cols: 0:16 sum128, 16:32 sumsq128, 32:48 mean, 48:64 ex2,
            # 64:80 -var, 80:96 rstd, 96:112 mean*rstd
            mean, ex2 = st[:, 32:48], st[:, 48:64]
            nvar, rstd, mrs = st[:, 64:80], st[:, 80:96], st[:, 96:112]
            nc.vector.tensor_scalar_mul(mean, st[:, 0:16], 1.0 / D)
            nc.vector.tensor_scalar_mul(ex2, st[:, 16:32], 1.0 / D)
            nc.vector.tensor_mul(nvar, mean, mean)
            nc.vector.tensor_sub(nvar, nvar, ex2)  # mean^2 - E[x^2] = -var
            nc.scalar.activation(
                rstd, nvar, mybir.ActivationFunctionType.Sqrt,
                bias=eps_sb[:, 0:1], scale=-1.0,
            )  # sqrt(var + eps)
            nc.vector.reciprocal(rstd, rstd)
            nc.vector.tensor_mul(mrs, mean, rstd)
            for ihalf in range(2):
                c8 = slice(ihalf * 8, (ihalf + 1) * 8)
                isl = slice(ihalf * 1024, (ihalf + 1) * 1024)
                nc.sync.dma_start(out=scr[0:1, isl], in_=rstd[:, c8])
                nc.sync.dma_start(out=scr[1:2, isl], in_=mrs[:, c8])
            # broadcast rows (DRAM -> 128 partitions)
            bcs = []
            for ihalf in range(2):
                isl = slice(ihalf * 1024, (ihalf + 1) * 1024)
                rstd_bc = bc_pool.tile([128, 1024], F32, name=f"rsbc_{b}_{ihalf}", tag="bc")
                nc.gpsimd.dma_start(out=rstd_bc, in_=scr[0:1, isl].partition_broadcast(128))
                mrs_bc = bc_pool.tile([128, 1024], F32, name=f"mrbc_{b}_{ihalf}", tag="bc")
                nc.gpsimd.dma_start(out=mrs_bc, in_=scr[1:2, isl].partition_broadcast(128))
                bcs.append((rstd_bc, mrs_bc))

            # ---- QKV projection on raw x; LN affine applied at eviction ----
            qT = qk_pool.tile([128, N], BF16, name=f"qT_{b}", tag=f"qT{b}")
            kT = qk_pool.tile([128, N], BF16, name=f"kT_{b}", tag=f"kT{b}")
            vT = vt_pool.tile([128, N], BF16, name=f"vT_{b}", tag="vT")
            qTs.append(qT)
            kTs.append(kT)
            sb_dst = [qT, kT, vT]
            for cc in (1, 2, 0):
                for ihalf in range(2):
                    isl = slice(ihalf * 1024, (ihalf + 1) * 1024)
                    pt = big_psum.tile([128, 1024], F32, name=f"qp_{b}_{cc}_{ihalf}", tag="big")
                    for kt in range(KT):
                        lhs = wqkv_sb[:, kt, cc * 128:(cc + 1) * 128]
                        for it2 in range(2):
                            s2 = slice(it2 * 512, (it2 + 1) * 512)
                            i2 = slice(ihalf * 1024 + it2 * 512, ihalf * 1024 + (it2 + 1) * 512)
                            bi = nc.tensor.matmul(
                                pt[:, s2], lhs, xts[kt][:, i2],
                                start=(kt == 0), stop=(kt == KT - 1),
                            )
                            if it2 == 1:
                                bi.ins.ldweights = False
                    rstd_bc, mrs_bc = bcs[ihalf]
                    tmp = tmp_pool.tile([128, 1024], F32, name=f"tmp_{b}_{cc}_{ihalf}", tag="tmp")
                    nc.vector.tensor_mul(tmp, pt, rstd_bc)
                    nc.vector.scalar_tensor_tensor(
                        out=sb_dst[cc][:, isl], in0=mrs_bc,
                        scalar=nwsum_sb[:, cc:cc + 1], in1=tmp,
                        op0=mybir.AluOpType.mult, op1=mybir.AluOpType.add,
                    )
                    nc.vector.tensor_scalar_add(
                        sb_dst[cc][:, isl], sb_dst[cc][:, isl], qkvb_sb[:, cc:cc + 1]
                    )

            # ---- v natural (+ ones column) via PE transpose ----
            vb = []
            for hh in range(HL):
                vn = vn_pool.tile([128, JC, DH + 1], BF16, name=f"vn_{b}_{hh}", tag=f"vn{b}{hh}")
                nc.gpsimd.memset(vn[:, :, DH:DH + 1], 1.0)
                vb.append(vn)
            vns.append(vb)
            for jc in range(JC):
                trp = big_psum.tile([128, 128], BF16, name=f"tr_{b}_{jc}", tag="big")
                nc.tensor.transpose(trp, vT[:, jc * 128:(jc + 1) * 128], ident)
                for hh in range(HL):
                    nc.vector.tensor_copy(
                        vb[hh][:, jc, 0:DH], trp[:, hh * DH:(hh + 1) * DH]
                    )

            ao_sb = ao_pool.tile([128, N], BF16, name=f"ao_{b}", tag=f"ao{b}")
            aos.append(ao_sb)

        # ---- attention: batch outer; ONE psum accumulator per group so two
        #      score tiles are in flight (PE never waits on ScalarE exp) ----
        scr3 = dscr_pool.tile([16, 1024], F32, name="scr3", tag="scr3")
        scr4 = dscr_pool.tile([16, 1024], F32, name="scr4", tag="scr4")
        for b in range(B):
            for hh in range(HL):
                hsl = slice(hh * DH, (hh + 1) * DH)
                for ihalf in range(2):
                    isl = slice(ihalf * 1024, (ihalf + 1) * 1024)
                    aop = ao_psum.tile([DH + 1, 1024], F32, name=f"aop_{b}_{hh}_{ihalf}", tag="aop")
                    for jc in range(JC):
                        jsl = slice(jc * 128, (jc + 1) * 128)
                        al_t = al_pool.tile([128, 1024], BF16, name=f"al_{b}_{hh}_{ihalf}_{jc}", tag="al")
                        nc.sync.dma_start(out=al_t, in_=al_d[hh, jsl, isl])
                        sc = big_psum.tile([128, 1024], F32, name=f"sc_{b}_{hh}_{ihalf}_{jc}", tag="big")
                        for it2 in range(2):
                            s2 = slice(it2 * 512, (it2 + 1) * 512)
                            i2 = slice(ihalf * 1024 + it2 * 512, ihalf * 1024 + (it2 + 1) * 512)
                            bi = nc.tensor.matmul(
                                sc[:, s2], kTs[b][hsl, jsl], qTs[b][hsl, i2],
                                start=True, stop=(it2 == 1),
                            )
                            if it2 == 1:
                                bi.ins.ldweights = False
                        nc.tensor.matmul(
                            sc[:, 0:512], ident, al_t[:, 0:512],
                            start=False, stop=True,
                        )
                        nc.vector.tensor_add(sc[:, 512:1024], sc[:, 512:1024], al_t[:, 512:1024])
                        at_t = at_pool.tile([128, 1024], BF16, name=f"at_{b}_{hh}_{ihalf}_{jc}", tag="at")
                        nc.scalar.activation(at_t, sc, mybir.ActivationFunctionType.Exp)
                        for it2 in range(2):
                            s2 = slice(it2 * 512, (it2 + 1) * 512)
                            bi = nc.tensor.matmul(
                                aop[:, s2], vns[b][hh][:, jc, :], at_t[:, s2],
                                start=(jc == 0), stop=(jc == JC - 1),
                            )
                            if it2 == 1:
                                bi.ins.ldweights = False
                    # evict raw attn output immediately (frees PSUM); the
                    # reciprocal runs at [128,8] via DMA reshapes and the
                    # normalize happens off the critical path
                    r = (hh * 2 + ihalf) * 2 + b
                    ao_raw = aor_pool.tile([DH + 1, 1024], F32, name=f"aor_{r}", tag="aor")
                    nc.vector.tensor_copy(ao_raw, aop)
                    nc.sync.dma_start(out=scr3[r:r + 1, :], in_=ao_raw[DH:DH + 1, :])
                    r128 = sm_pool.tile([128, 8], F32, name=f"r128_{r}", tag="r128", bufs=3)
                    nc.sync.dma_start(out=r128, in_=scr3[r:r + 1, :])
                    nc.vector.reciprocal(r128, r128)
                    nc.sync.dma_start(out=scr4[r:r + 1, :], in_=r128)
                    rr_bc = rrbc_pool.tile([DH, 1024], F32, name=f"rrbc_{r}", tag="rrbc")
                    nc.gpsimd.dma_start(
                        out=rr_bc, in_=scr4[r:r + 1, :].partition_broadcast(DH)
                    )
                    nc.vector.tensor_mul(aos[b][hsl, isl], ao_raw[0:DH, :], rr_bc)

        # ---- out projection (partial, transposed, bf16) ----
        for ihalf in range(2):
            for b in range(B):
                for ec in range(8):
                    lhs = wout_sb[:, ec * 128:(ec + 1) * 128]
                    isl = slice(ihalf * 1024, (ihalf + 1) * 1024)
                    opp = big_psum.tile([128, 1024], F32, name=f"op_{b}_{ec}_{ihalf}", tag="big")
                    for it2 in range(2):
                        s2 = slice(it2 * 512, (it2 + 1) * 512)
                        i2 = slice(ihalf * 1024 + it2 * 512, ihalf * 1024 + (it2 + 1) * 512)
                        bi = nc.tensor.matmul(opp[:, s2], lhs, aos[b][:, i2], start=True, stop=True)
                        if it2 == 1:
                            bi.ins.ldweights = False
                    ob = ob_pool.tile([128, 1024], BF16, name=f"ob_{b}_{ec}_{ihalf}", tag="ob")
                    nc.vector.tensor_copy(ob, opp)
                    nc.sync.dma_start(out=out_d[b, ec * 128:(ec + 1) * 128, isl], in_=ob)
    nc.compile()
    return nc


def make_in_maps(x, alibi_bias, ln_gamma, ln_beta, w_qkv, w_out):
    """Host-side sharding / layout prep. Returns list of 8 per-core input dicts."""
    x = np.asarray(x, np.float32)
    alibi_bias = np.asarray(alibi_bias, np.float32)
    ln_gamma = np.asarray(ln_gamma, np.float32)
    ln_beta = np.asarray(ln_beta, np.float32)
    w_qkv = np.asarray(w_qkv, np.float32)
    w_out = np.asarray(w_out, np.float32)
    BF = ml_dtypes.bfloat16

    xt = np.ascontiguousarray(x.transpose(0, 2, 1)).astype(BF)  # [B, D, N]
    # fold ln_gamma into w_qkv rows; fold attention scale into the q columns
    w_eff = w_qkv * ln_gamma[:, None]
    qkvb_full = ln_beta @ w_qkv  # [3*H*DH]
    in_maps = []
    for c in range(NCORES):
        csl = slice(c * CL, (c + 1) * CL)
        wq = w_eff[:, 0:H * DH][:, csl] * SCALE
        wk = w_eff[:, H * DH:2 * H * DH][:, csl]
        wv = w_eff[:, 2 * H * DH:3 * H * DH][:, csl]
        wqkv_c = np.ascontiguousarray(np.concatenate([wq, wk, wv], axis=1)).astype(BF)
        nwsum_c = -wqkv_c.astype(np.float64).sum(axis=0).astype(np.float32)
        qb = qkvb_full.reshape(3, H * DH)[:, csl].copy()
        qb[0] *= SCALE
        qkvb_c = np.ascontiguousarray(qb.reshape(-1))
        al_c = np.ascontiguousarray(
            alibi_bias[0, c * HL:(c + 1) * HL].transpose(0, 2, 1)
        ).astype(BF)
        wout_c = np.ascontiguousarray(w_out[csl, :]).astype(BF)
        in_maps.append({
            "xt": xt,
            "alibi": al_c,
            "wqkv": wqkv_c,
            "qkvb": qkvb_c,
            "nwsum": nwsum_c,
            "wout": wout_c,
            "ones": np.ones((128, 1), BF),
        })
    return in_maps


def kernel(x, alibi_bias, mask, ln_gamma, ln_beta, w_qkv, w_out, _trace=False):
    global _CACHED_NC
    mask = np.asarray(mask)
    assert mask.all(), "kernel assumes an all-True mask"
    if _CACHED_NC is None:
        _CACHED_NC = build_nc()
    nc = _CACHED_NC
    in_maps = make_in_maps(x, alibi_bias, ln_gamma, ln_beta, w_qkv, w_out)
    res = run_bass_kernel_spmd(nc, in_maps, core_ids=list(range(NCORES)), trace=_trace)
    out_t = np.zeros((B, D, N), np.float32)
    for c in range(NCORES):
        out_t += res.results[c]["out"].astype(np.float32)
    out = np.ascontiguousarray(out_t.transpose(0, 2, 1))
    if _trace:
        return out, res
    return out


# revision 51
# speedup vs baseline: 1.1353x; 1.1353x over previous
"""Trainium2 Bass kernel for nn_Attention (LN -> QKV -> alibi attention -> out-proj).

Full shapes: x[2,2048,1024], alibi[1,16,2048,2048], w_qkv[1024,3072], w_out[1024,1024].
Sharding: tensor-parallel over heads. Core c owns heads {2c, 2c+1} for BOTH batches.
Each core computes a partial out-projection; the host sums the 8 partials (the
tensor-parallel reduction) and transposes back.

v2 design (all matmuls bf16 -- fp32r streams at 2 cyc/row on silicon, bf16 at 1):
  - x passed host-transposed + bf16: xT[b] = [d=1024, i=2048].
  - LN folded into the QKV eviction: qkv = rstd*(W^T x) + (mean*rstd)*(-colsum(W))
    (+ beta@W). LN stats (sum, sum-sq) via matmul-with-ones run concurrently with
    the QKV matmuls on raw x; no xn materialization, no LN->QKV serialization.
  - q/k evicted bf16 (2 heads on partitions); v bf16, PE-transposed to v-natural
    [j, 64d + ones-col]; the ones column makes attn@v also emit softmax denoms.
  - attention loops h outer, batch inner: each alibi^T tile (bf16, host-transposed)
    is DMA'd once and used by both batches (16MB/core alibi traffic, the minimum).
  - scores S^T = kT_chunk^T @ qT (K=64) + identity-matmul alibi accumulate; exp on
    ScalarE (PSUM f32 -> SBUF bf16), no max-subtraction (|scores| <~ 15).
  - PSUM: one shared [128,1024] pool (bufs=2) for stats/qkv/transpose/scores/
    out-proj + one [65,1024] pool (bufs=2) holding both batches' attn accumulators.
  - out partials written bf16 transposed [b, e, i]; host sums in f32.
"""

import sys

sys.path.insert(0, "/opt/trn_rl_repo")

from contextlib import ExitStack

import numpy as np
import ml_dtypes

import concourse.bass as bass
from concourse import bacc
import concourse.mybir as mybir
import concourse.tile as tile
from concourse.bass_utils import run_bass_kernel_spmd
from concourse.masks import make_identity

F32 = mybir.dt.float32
BF16 = mybir.dt.bfloat16

B, N, D = 2, 2048, 1024
H, DH = 16, 64
NCORES = 8
HL = H // NCORES          # local heads per core = 2
CL = HL * DH              # local head channels = 128
LN_EPS = 1e-5
SCALE = DH ** -0.5
KT = D // 128             # 8 d-tiles
JC = N // 128             # 16 j-chunks
IT = N // 512             # 4 i-tiles of 512

_CACHED_NC = None


def build_nc() -> bass.Bass:
    nc = bacc.Bacc(None)
    xt_d = nc.declare_dram_parameter("xt", [B, D, N], BF16, isOutput=False)
    al_d = nc.declare_dram_parameter("alibi", [HL, N, N], BF16, isOutput=False)
    wqkv_d = nc.declare_dram_parameter("wqkv", [D, 3 * CL], BF16, isOutput=False)
    qkvb_d = nc.declare_dram_parameter("qkvb", [3 * CL], F32, isOutput=False)
    nwsum_d = nc.declare_dram_parameter("nwsum", [3 * CL], F32, isOutput=False)
    wout_d = nc.declare_dram_parameter("wout", [CL, D], BF16, isOutput=False)
    ones_d = nc.declare_dram_parameter("ones", [128, 1], BF16, isOutput=False)
    out_d = nc.declare_dram_parameter("out", [B, D, N], BF16, isOutput=True)

    with tile.TileContext(nc) as tc, ExitStack() as ctx:
        ep = lambda **kw: ctx.enter_context(tc.tile_pool(**kw))
        cpool = ep(name="const", bufs=1)
        xt_pool = ep(name="xt", bufs=11)
        sq_pool = ep(name="sq", bufs=4)
        sm_pool = ep(name="small", bufs=2)
        tmp_pool = ep(name="tmp", bufs=4)
        qk_pool = ep(name="qk", bufs=1)      # per-batch tiles, all resident
        vt_pool = ep(name="vt", bufs=2)
        vn_pool = ep(name="vn", bufs=1)      # 4 resident tiles (b x head)
        al_pool = ep(name="al", bufs=8)
        at_pool = ep(name="at", bufs=8)
        ao_pool = ep(name="aos", bufs=1)
        ob_pool = ep(name="ob", bufs=4)
        bc_pool = ep(name="bc", bufs=4)
        rrbc_pool = ep(name="rrbc", bufs=3)
        aor_pool = ep(name="aor", bufs=3)
        dscr_pool = ep(name="dscr", bufs=2, space="DRAM")
        big_psum = ep(name="ps_big", bufs=2, space="PSUM")
        ao_psum = ep(name="ps_ao", bufs=2, space="PSUM")

        # ---- constants ----
        ident = cpool.tile([128, 128], BF16, name="ident")
        make_identity(nc, ident)
        zero_sb = cpool.tile([128, 1], F32, name="zero_sb")
        nc.vector.memset(zero_sb, 0.0)
        nc.const_aps.aps[(F32, 0.0)] = zero_sb[:, 0:1]
        eps_sb = cpool.tile([128, 1], F32, name="eps_sb")
        nc.vector.memset(eps_sb, LN_EPS)
        ones_sb = cpool.tile([128, 1], BF16, name="ones_sb")
        nc.sync.dma_start(out=ones_sb, in_=ones_d[:, :])
        wqkv_sb = cpool.tile([128, KT, 3 * CL], BF16, name="wqkv_sb")
        nc.sync.dma_start(out=wqkv_sb, in_=wqkv_d.rearrange("(t p) c -> p t c", p=128))
        qkvb_sb = cpool.tile([128, 3], F32, name="qkvb_sb")
        nc.sync.dma_start(out=qkvb_sb, in_=qkvb_d.rearrange("(c p) -> p c", p=128))
        nwsum_sb = cpool.tile([128, 3], F32, name="nwsum_sb")
        nc.sync.dma_start(out=nwsum_sb, in_=nwsum_d.rearrange("(c p) -> p c", p=128))
        wout_sb = cpool.tile([128, D], BF16, name="wout_sb")
        nc.sync.dma_start(out=wout_sb, in_=wout_d[:, :])

        qTs, kTs, vns, aos = [], [], [], []
        for b in range(B):
            # ---- load xT (bf16) ----
            xts = []
            for kt in range(KT):
                xt_t = xt_pool.tile([128, N], BF16, name=f"xt_{b}_{kt}", tag="xt")
                nc.sync.dma_start(out=xt_t, in_=xt_d[b, kt * 128:(kt + 1) * 128, :])
                xts.append(xt_t)

            # ---- LN stats (sum, sumsq) via matmul-with-ones ----

            scr = dscr_pool.tile([2, N], F32, name=f"scr_{b}", tag="scr")
            st = sm_pool.tile([128, 112], F32, name=f"st_{b}", tag="st128")
            for ihalf in range(2):
                isl = slice(ihalf * 1024, (ihalf + 1) * 1024)
                rows = sm_pool.tile([1, N], F32, name=f"rows_{b}_{ihalf}", tag="rows", bufs=1)
                sum_ps = big_psum.tile([1, 1024], F32, name=f"sum_{b}_{ihalf}", tag="big")
                sq_ps = big_psum.tile([33, 1024], F32, name=f"ssq_{b}_{ihalf}", tag="big")
                for kt in range(KT):
                    xsq = sq_pool.tile([128, 1024], BF16, name=f"xsq_{b}_{ihalf}_{kt}", tag="sq")
                    nc.gpsimd.tensor_mul(xsq, xts[kt][:, isl], xts[kt][:, isl])
                    for it2 in range(2):
                        s2 = slice(it2 * 512, (it2 + 1) * 512)
                        i2 = slice(ihalf * 1024 + it2 * 512, ihalf * 1024 + (it2 + 1) * 512)
                        nc.tensor.matmul(
                            sum_ps[0:1, s2], ones_sb, xts[kt][:, i2],
                            start=(kt == 0), stop=(kt == KT - 1),
                        )
                        nc.tensor.matmul(
                            sq_ps[32:33, s2], ones_sb, xsq[:, s2],
                            start=(kt == 0), stop=(kt == KT - 1),
                            tile_position=(0, 32),
                        )
                nc.vector.tensor_copy(rows[0:1, 0:1024], sum_ps)
                nc.vector.tensor_copy(rows[0:1, 1024:2048], sq_ps[32:33, :])
                nc.sync.dma_start(out=st[:, ihalf * 8:(ihalf + 1) * 8], in_=rows[0:1, 0:1024])
                nc.sync.dma_start(out=st[:, 16 + ihalf * 8:16 + (ihalf + 1) * 8], in_=rows[0:1, 1024:2048])
            # stat cols: 0:16 sum128, 16:32 sumsq128, 32:48 mean, 48:64 ex2,
            # 64:80 -var, 80:96 rstd, 96:112 mean*rstd
            mean, ex2 = st[:, 32:48], st[:, 48:64]
            nvar, rstd, mrs = st[:, 64:80], st[:, 80:96], st[:, 96:112]
            nc.vector.tensor_scalar_mul(mean, st[:, 0:16], 1.0 / D)
            nc.vector.tensor_scalar_mul(ex2, st[:, 16:32], 1.0 / D)
            nc.vector.tensor_mul(nvar, mean, mean)
            nc.vector.tensor_sub(nvar, nvar, ex2)  # mean^2 - E[x^2] = -var
            nc.scalar.activation(
                rstd, nvar, mybir.ActivationFunctionType.Sqrt,
                bias=eps_sb[:, 0:1], scale=-1.0,
            )  # sqrt(var + eps)
            nc.vector.reciprocal(rstd, rstd)
            nc.vector.tensor_mul(mrs, mean, rstd)
            for ihalf in range(2):
                c8 = slice(ihalf * 8, (ihalf + 1) * 8)
                isl = slice(ihalf * 1024, (ihalf + 1) * 1024)
                nc.sync.dma_start(out=scr[0:1, isl], in_=rstd[:, c8])
                nc.sync.dma_start(out=scr[1:2, isl], in_=mrs[:, c8])
            # broadcast rows (DRAM -> 128 partitions)
            bcs = []
            for ihalf in range(2):
                isl = slice(ihalf * 1024, (ihalf + 1) * 1024)
                rstd_bc = bc_pool.tile([128, 1024], F32, name=f"rsbc_{b}_{ihalf}", tag="bc")
                nc.gpsimd.dma_start(out=rstd_bc, in_=scr[0:1, isl].partition_broadcast(128))
                mrs_bc = bc_pool.tile([128, 1024], F32, name=f"mrbc_{b}_{ihalf}", tag="bc")
                nc.gpsimd.dma_start(out=mrs_bc, in_=scr[1:2, isl].partition_broadcast(128))
                bcs.append((rstd_bc, mrs_bc))

            # ---- QKV projection on raw x; LN affine applied at eviction ----
            qT = qk_pool.tile([128, N], BF16, name=f"qT_{b}", tag=f"qT{b}")
            kT = qk_pool.tile([128, N], BF16, name=f"kT_{b}", tag=f"kT{b}")
            vT = vt_pool.tile([128, N], BF16, name=f"vT_{b}", tag="vT")
            qTs.append(qT)
            kTs.append(kT)
            sb_dst = [qT, kT, vT]
            for cc in (1, 2, 0):
                for ihalf in range(2):
                    isl = slice(ihalf * 1024, (ihalf + 1) * 1024)
                    pt = big_psum.tile([128, 1024], F32, name=f"qp_{b}_{cc}_{ihalf}", tag="big")
                    for kt in range(KT):
                        lhs = wqkv_sb[:, kt, cc * 128:(cc + 1) * 128]
                        for it2 in range(2):
                            s2 = slice(it2 * 512, (it2 + 1) * 512)
                            i2 = slice(ihalf * 1024 + it2 * 512, ihalf * 1024 + (it2 + 1) * 512)
                            bi = nc.tensor.matmul(
                                pt[:, s2], lhs, xts[kt][:, i2],
                                start=(kt == 0), stop=(kt == KT - 1),
                            )
                            if it2 == 1:
                                bi.ins.ldweights = False
                    rstd_bc, mrs_bc = bcs[ihalf]
                    tmp = tmp_pool.tile([128, 1024], F32, name=f"tmp_{b}_{cc}_{ihalf}", tag="tmp")
                    nc.vector.tensor_mul(tmp, pt, rstd_bc)
                    nc.vector.scalar_tensor_tensor(
                        out=sb_dst[cc][:, isl], in0=mrs_bc,
                        scalar=nwsum_sb[:, cc:cc + 1], in1=tmp,
                        op0=mybir.AluOpType.mult, op1=mybir.AluOpType.add,
                    )
                    nc.vector.tensor_scalar_add(
                        sb_dst[cc][:, isl], sb_dst[cc][:, isl], qkvb_sb[:, cc:cc + 1]
                    )

            # ---- v natural (+ ones column) via PE transpose ----
            vb = []
            for hh in range(HL):
                vn = vn_pool.tile([128, JC, DH + 1], BF16, name=f"vn_{b}_{hh}", tag=f"vn{b}{hh}")
                nc.gpsimd.memset(vn[:, :, DH:DH + 1], 1.0)
                vb.append(vn)
            vns.append(vb)
            for jc in range(JC):
                trp = big_psum.tile([128, 128], BF16, name=f"tr_{b}_{jc}", tag="big")
                nc.tensor.transpose(trp, vT[:, jc * 128:(jc + 1) * 128], ident)
                for hh in range(HL):
                    nc.vector.tensor_copy(
                        vb[hh][:, jc, 0:DH], trp[:, hh * DH:(hh + 1) * DH]
                    )

            ao_sb = ao_pool.tile([128, N], BF16, name=f"ao_{b}", tag=f"ao{b}")
            aos.append(ao_sb)

        # ---- attention: batch outer; ONE psum accumulator per group so two
        #      score tiles are in flight (PE never waits on ScalarE exp) ----
        scr3 = dscr_pool.tile([16, 1024], F32, name="scr3", tag="scr3")
        scr4 = dscr_pool.tile([16, 1024], F32, name="scr4", tag="scr4")
        for b in range(B):
            for hh in range(HL):
                hsl = slice(hh * DH, (hh + 1) * DH)
                for ihalf in range(2):
                    isl = slice(ihalf * 1024, (ihalf + 1) * 1024)
                    aop = ao_psum.tile([DH + 1, 1024], F32, name=f"aop_{b}_{hh}_{ihalf}", tag="aop")
                    for jc in range(JC):
                        jsl = slice(jc * 128, (jc + 1) * 128)
                        al_t = al_pool.tile([128, 1024], BF16, name=f"al_{b}_{hh}_{ihalf}_{jc}", tag="al")
                        nc.sync.dma_start(out=al_t, in_=al_d[hh, jsl, isl])
                        sc = big_psum.tile([128, 1024], F32, name=f"sc_{b}_{hh}_{ihalf}_{jc}", tag="big")
                        for it2 in range(2):
                            s2 = slice(it2 * 512, (it2 + 1) * 512)
                            i2 = slice(ihalf * 1024 + it2 * 512, ihalf * 1024 + (it2 + 1) * 512)
                            bi = nc.tensor.matmul(
                                sc[:, s2], kTs[b][hsl, jsl], qTs[b][hsl, i2],
                                start=True, stop=(it2 == 1),
                            )
                            if it2 == 1:
                                bi.ins.ldweights = False
                        nc.tensor.matmul(
                            sc[:, 0:512], ident, al_t[:, 0:512],
                            start=False, stop=True,
                        )
                        nc.vector.tensor_add(sc[:, 512:1024], sc[:, 512:1024], al_t[:, 512:1024])
                        at_t = at_pool.tile([128, 1024], BF16, name=f"at_{b}_{hh}_{ihalf}_{jc}", tag="at")
                        nc.scalar.activation(at_t, sc, mybir.ActivationFunctionType.Exp)
                        for it2 in range(2):
                            s2 = slice(it2 * 512, (it2 + 1) * 512)
                            bi = nc.tensor.matmul(
                                aop[:, s2], vns[b][hh][:, jc, :], at_t[:, s2],
                                start=(jc == 0), stop=(jc == JC - 1),
                            )
                            if it2 == 1:
                                bi.ins.ldweights = False
                    # evict raw attn output immediately (frees PSUM); the
                    # reciprocal runs at [128,8] via DMA reshapes and the
                    # normalize happens off the critical path
                    r = (hh * 2 + ihalf) * 2 + b
                    ao_raw = aor_pool.tile([DH + 1, 1024], F32, name=f"aor_{r}", tag="aor")
                    nc.vector.tensor_copy(ao_raw, aop)
                    nc.sync.dma_start(out=scr3[r:r + 1, :], in_=ao_raw[DH:DH + 1, :])
                    r128 = sm_pool.tile([128, 8], F32, name=f"r128_{r}", tag="r128", bufs=3)
                    nc.sync.dma_start(out=r128, in_=scr3[r:r + 1, :])
                    nc.vector.reciprocal(r128, r128)
                    nc.sync.dma_start(out=scr4[r:r + 1, :], in_=r128)
                    rr_bc = rrbc_pool.tile([DH, 1024], F32, name=f"rrbc_{r}", tag="rrbc")
                    nc.gpsimd.dma_start(
                        out=rr_bc, in_=scr4[r:r + 1, :].partition_broadcast(DH)
                    )
                    nc.vector.tensor_mul(aos[b][hsl, isl], ao_raw[0:DH, :], rr_bc)

        # ---- out projection (partial, transposed, bf16) ----
        for ihalf in range(2):
            for b in range(B):
                for ec in range(8):
                    lhs = wout_sb[:, ec * 128:(ec + 1) * 128]
                    isl = slice(ihalf * 1024, (ihalf + 1) * 1024)
                    opp = big_psum.tile([128, 1024], F32, name=f"op_{b}_{ec}_{ihalf}", tag="big")
                    for it2 in range(2):
                        s2 = slice(it2 * 512, (it2 + 1) * 512)
                        i2 = slice(ihalf * 1024 + it2 * 512, ihalf * 1024 + (it2 + 1) * 512)
                        bi = nc.tensor.matmul(opp[:, s2], lhs, aos[b][:, i2], start=True, stop=True)
                        if it2 == 1:
                            bi.ins.ldweights = False
                    ob = ob_pool.tile([128, 1024], BF16, name=f"ob_{b}_{ec}_{ihalf}", tag="ob")
                    nc.vector.tensor_copy(ob, opp)
                    nc.sync.dma_start(out=out_d[b, ec * 128:(ec + 1) * 128, isl], in_=ob)
    nc.compile()
    return nc


def make_in_maps(x, alibi_bias, ln_gamma, ln_beta, w_qkv, w_out):
    """Host-side sharding / layout prep. Returns list of 8 per-core input dicts."""
    x = np.asarray(x, np.float32)
    alibi_bias = np.asarray(alibi_bias, np.float32)
    ln_gamma = np.asarray(ln_gamma, np.float32)
    ln_beta = np.asarray(ln_beta, np.float32)
    w_qkv = np.asarray(w_qkv, np.float32)
    w_out = np.asarray(w_out, np.float32)
    BF = ml_dtypes.bfloat16

    xt = np.ascontiguousarray(x.transpose(0, 2, 1)).astype(BF)  # [B, D, N]
    # fold ln_gamma into w_qkv rows; fold attention scale into the q columns
    w_eff = w_qkv * ln_gamma[:, None]
    qkvb_full = ln_beta @ w_qkv  # [3*H*DH]
    in_maps = []
    for c in range(NCORES):
        csl = slice(c * CL, (c + 1) * CL)
        wq = w_eff[:, 0:H * DH][:, csl] * SCALE
        wk = w_eff[:, H * DH:2 * H * DH][:, csl]
        wv = w_eff[:, 2 * H * DH:3 * H * DH][:, csl]
        wqkv_c = np.ascontiguousarray(np.concatenate([wq, wk, wv], axis=1)).astype(BF)
        nwsum_c = -wqkv_c.astype(np.float64).sum(axis=0).astype(np.float32)
        qb = qkvb_full.reshape(3, H * DH)[:, csl].copy()
        qb[0] *= SCALE
        qkvb_c = np.ascontiguousarray(qb.reshape(-1))
        al_c = np.ascontiguousarray(
            alibi_bias[0, c * HL:(c + 1) * HL].transpose(0, 2, 1)
        ).astype(BF)
        wout_c = np.ascontiguousarray(w_out[csl, :]).astype(BF)
        in_maps.append({
            "xt": xt,
            "alibi": al_c,
            "wqkv": wqkv_c,
            "qkvb": qkvb_c,
            "nwsum": nwsum_c,
            "wout": wout_c,
            "ones": np.ones((128, 1), BF),
        })
    return in_maps


def kernel(x, alibi_bias, mask, ln_gamma, ln_beta, w_qkv, w_out, _trace=False):
    global _CACHED_NC
    mask = np.asarray(mask)
    assert mask.all(), "kernel assumes an all-True mask"
    if _CACHED_NC is None:
        _CACHED_NC = build_nc()
    nc = _CACHED_NC
    in_maps = make_in_maps(x, alibi_bias, ln_gamma, ln_beta, w_qkv, w_out)
    res = run_bass_kernel_spmd(nc, in_maps, core_ids=list(range(NCORES)), trace=_trace)
    out_t = np.zeros((B, D, N), np.float32)
    for c in range(NCORES):
        out_t += res.results[c]["out"].astype(np.float32)
    out = np.ascontiguousarray(out_t.transpose(0, 2, 1))
    if _trace:
        return out, res
    return out


# revision 55
# speedup vs baseline: 1.2468x; 1.0982x over previous
"""Trainium2 Bass kernel for nn_Attention (LN -> QKV -> alibi attention -> out-proj).

Full shapes: x[2,2048,1024], alibi[1,16,2048,2048], w_qkv[1024,3072], w_out[1024,1024].
Sharding: tensor-parallel over heads. Core c owns heads {2c, 2c+1} for BOTH batches.
Each core computes a partial out-projection; the host sums the 8 partials (the
tensor-parallel reduction) and transposes back.

v2 design (all matmuls bf16 -- fp32r streams at 2 cyc/row on silicon, bf16 at 1):
  - x passed host-transposed + bf16: xT[b] = [d=1024, i=2048].
  - LN folded into the QKV eviction: qkv = rstd*(W^T x) + (mean*rstd)*(-colsum(W))
    (+ beta@W). LN stats (sum, sum-sq) via matmul-with-ones run concurrently with
    the QKV matmuls on raw x; no xn materialization, no LN->QKV serialization.
  - q/k evicted bf16 (2 heads on partitions); v bf16, PE-transposed to v-natural
    [j, 64d + ones-col]; the ones column makes attn@v also emit softmax denoms.
  - attention loops h outer, batch inner: each alibi^T tile (bf16, host-transposed)
    is DMA'd once and used by both batches (16MB/core alibi traffic, the minimum).
  - scores S^T = kT_chunk^T @ qT (K=64) + identity-matmul alibi accumulate; exp on
    ScalarE (PSUM f32 -> SBUF bf16), no max-subtraction (|scores| <~ 15).
  - PSUM: one shared [128,1024] pool (bufs=2) for stats/qkv/transpose/scores/
    out-proj + one [65,1024] pool (bufs=2) holding both batches' attn accumulators.
  - out partials written bf16 transposed [b, e, i]; host sums in f32.
"""

import sys

sys.path.insert(0, "/opt/trn_rl_repo")

from contextlib import ExitStack

import numpy as np
import ml_dtypes

import concourse.bass as bass
from concourse import bacc
import concourse.mybir as mybir
import concourse.tile as tile
from concourse.bass_utils import run_bass_kernel_spmd
from concourse.masks import make_identity

F32 = mybir.dt.float32
BF16 = mybir.dt.bfloat16

B, N, D = 2, 2048, 1024
H, DH = 16, 64
NCORES = 8
HL = H // NCORES          # local heads per core = 2
CL = HL * DH              # local head channels = 128
LN_EPS = 1e-5
SCALE = DH ** -0.5
KT = D // 128             # 8 d-tiles
JC = N // 128             # 16 j-chunks
IT = N // 512             # 4 i-tiles of 512

_CACHED_NC = None


def build_nc() -> bass.Bass:
    nc = bacc.Bacc(None)
    xt_d = nc.declare_dram_parameter("xt", [B, D, N], BF16, isOutput=False)
    al_d = nc.declare_dram_parameter("alibi", [HL, N, N], BF16, isOutput=False)
    wqkv_d = nc.declare_dram_parameter("wqkv", [D, 3 * CL], BF16, isOutput=False)
    qkvb_d = nc.declare_dram_parameter("qkvb", [3 * CL], F32, isOutput=False)
    nwsum_d = nc.declare_dram_parameter("nwsum", [3 * CL], F32, isOutput=False)
    wout_d = nc.declare_dram_parameter("wout", [CL, D], BF16, isOutput=False)
    ones_d = nc.declare_dram_parameter("ones", [128, 1], BF16, isOutput=False)
    out_d = nc.declare_dram_parameter("out", [B, D, N], BF16, isOutput=True)

    with tile.TileContext(nc) as tc, ExitStack() as ctx:
        ep = lambda **kw: ctx.enter_context(tc.tile_pool(**kw))
        cpool = ep(name="const", bufs=1)
        xt_pool = ep(name="xt", bufs=11)
        sq_pool = ep(name="sq", bufs=4)
        sm_pool = ep(name="small", bufs=2)
        tmp_pool = ep(name="tmp", bufs=4)
        qk_pool = ep(name="qk", bufs=1)      # per-batch tiles, all resident
        vt_pool = ep(name="vt", bufs=2)
        vn_pool = ep(name="vn", bufs=1)      # 4 resident tiles (b x head)
        al_pool = ep(name="al", bufs=8)
        at_pool = ep(name="at", bufs=8)
        ao_pool = ep(name="aos", bufs=1)
        ob_pool = ep(name="ob", bufs=4)
        bc_pool = ep(name="bc", bufs=4)
        rrbc_pool = ep(name="rrbc", bufs=3)
        aor_pool = ep(name="aor", bufs=3)
        dscr_pool = ep(name="dscr", bufs=2, space="DRAM")
        big_psum = ep(name="ps_big", bufs=3, space="PSUM")
        ao_psum = ep(name="ps_ao", bufs=1, space="PSUM")

        # ---- constants ----
        ident = cpool.tile([128, 128], BF16, name="ident")
        make_identity(nc, ident)
        zero_sb = cpool.tile([128, 1], F32, name="zero_sb")
        nc.vector.memset(zero_sb, 0.0)
        nc.const_aps.aps[(F32, 0.0)] = zero_sb[:, 0:1]
        eps_sb = cpool.tile([128, 1], F32, name="eps_sb")
        nc.vector.memset(eps_sb, LN_EPS)
        ones_sb = cpool.tile([128, 1], BF16, name="ones_sb")
        nc.sync.dma_start(out=ones_sb, in_=ones_d[:, :])
        wqkv_sb = cpool.tile([128, KT, 3 * CL], BF16, name="wqkv_sb")
        nc.sync.dma_start(out=wqkv_sb, in_=wqkv_d.rearrange("(t p) c -> p t c", p=128))
        qkvb_sb = cpool.tile([128, 3], F32, name="qkvb_sb")
        nc.sync.dma_start(out=qkvb_sb, in_=qkvb_d.rearrange("(c p) -> p c", p=128))
        nwsum_sb = cpool.tile([128, 3], F32, name="nwsum_sb")
        nc.sync.dma_start(out=nwsum_sb, in_=nwsum_d.rearrange("(c p) -> p c", p=128))
        wout_sb = cpool.tile([128, D], BF16, name="wout_sb")
        nc.sync.dma_start(out=wout_sb, in_=wout_d[:, :])

        qTs, kTs, vns, aos = [], [], [], []
        for b in range(B):
            # ---- load xT (bf16) ----
            xts = []
            for kt in range(KT):
                xt_t = xt_pool.tile([128, N], BF16, name=f"xt_{b}_{kt}", tag="xt")
                nc.sync.dma_start(out=xt_t, in_=xt_d[b, kt * 128:(kt + 1) * 128, :])
                xts.append(xt_t)

            # ---- LN stats (sum, sumsq) via matmul-with-ones ----

            scr = dscr_pool.tile([2, N], F32, name=f"scr_{b}", tag="scr")
            st = sm_pool.tile([128, 112], F32, name=f"st_{b}", tag="st128")
            for ihalf in range(2):
                isl = slice(ihalf * 1024, (ihalf + 1) * 1024)
                rows = sm_pool.tile([1, N], F32, name=f"rows_{b}_{ihalf}", tag="rows", bufs=1)
                sum_ps = big_psum.tile([1, 1024], F32, name=f"sum_{b}_{ihalf}", tag="big")
                sq_ps = big_psum.tile([33, 1024], F32, name=f"ssq_{b}_{ihalf}", tag="big")
                for kt in range(KT):
                    xsq = sq_pool.tile([128, 1024], BF16, name=f"xsq_{b}_{ihalf}_{kt}", tag="sq")
                    nc.gpsimd.tensor_mul(xsq, xts[kt][:, isl], xts[kt][:, isl])
                    for it2 in range(2):
                        s2 = slice(it2 * 512, (it2 + 1) * 512)
                        i2 = slice(ihalf * 1024 + it2 * 512, ihalf * 1024 + (it2 + 1) * 512)
                        nc.tensor.matmul(
                            sum_ps[0:1, s2], ones_sb, xts[kt][:, i2],
                            start=(kt == 0), stop=(kt == KT - 1),
                        )
                        nc.tensor.matmul(
                            sq_ps[32:33, s2], ones_sb, xsq[:, s2],
                            start=(kt == 0), stop=(kt == KT - 1),
                            tile_position=(0, 32),
                        )
                nc.vector.tensor_copy(rows[0:1, 0:1024], sum_ps)
                nc.vector.tensor_copy(rows[0:1, 1024:2048], sq_ps[32:33, :])
                nc.sync.dma_start(out=st[:, ihalf * 8:(ihalf + 1) * 8], in_=rows[0:1, 0:1024])
                nc.sync.dma_start(out=st[:, 16 + ihalf * 8:16 + (ihalf + 1) * 8], in_=rows[0:1, 1024:2048])
            # stat cols: 0:16 sum128, 16:32 sumsq128, 32:48 mean, 48:64 ex2,
            # 64:80 -var, 80:96 rstd, 96:112 mean*rstd
            mean, ex2 = st[:, 32:48], st[:, 48:64]
            nvar, rstd, mrs = st[:, 64:80], st[:, 80:96], st[:, 96:112]
            nc.vector.tensor_scalar_mul(mean, st[:, 0:16], 1.0 / D)
            nc.vector.tensor_scalar_mul(ex2, st[:, 16:32], 1.0 / D)
            nc.vector.tensor_mul(nvar, mean, mean)
            nc.vector.tensor_sub(nvar, nvar, ex2)  # mean^2 - E[x^2] = -var
            nc.scalar.activation(
                rstd, nvar, mybir.ActivationFunctionType.Sqrt,
                bias=eps_sb[:, 0:1], scale=-1.0,
            )  # sqrt(var + eps)
            nc.vector.reciprocal(rstd, rstd)
            nc.vector.tensor_mul(mrs, mean, rstd)
            for ihalf in range(2):
                c8 = slice(ihalf * 8, (ihalf + 1) * 8)
                isl = slice(ihalf * 1024, (ihalf + 1) * 1024)
                nc.sync.dma_start(out=scr[0:1, isl], in_=rstd[:, c8])
                nc.sync.dma_start(out=scr[1:2, isl], in_=mrs[:, c8])
            # broadcast rows (DRAM -> 128 partitions)
            bcs = []
            for ihalf in range(2):
                isl = slice(ihalf * 1024, (ihalf + 1) * 1024)
                rstd_bc = bc_pool.tile([128, 1024], F32, name=f"rsbc_{b}_{ihalf}", tag="bc")
                nc.gpsimd.dma_start(out=rstd_bc, in_=scr[0:1, isl].partition_broadcast(128))
                mrs_bc = bc_pool.tile([128, 1024], F32, name=f"mrbc_{b}_{ihalf}", tag="bc")
                nc.gpsimd.dma_start(out=mrs_bc, in_=scr[1:2, isl].partition_broadcast(128))
                bcs.append((rstd_bc, mrs_bc))

            # ---- QKV projection on raw x; LN affine applied at eviction ----
            qT = qk_pool.tile([128, N], BF16, name=f"qT_{b}", tag=f"qT{b}")
            kT = qk_pool.tile([128, N], BF16, name=f"kT_{b}", tag=f"kT{b}")
            vT = vt_pool.tile([128, N], BF16, name=f"vT_{b}", tag="vT")
            qTs.append(qT)
            kTs.append(kT)
            sb_dst = [qT, kT, vT]
            for cc in (1, 2, 0):
                for ihalf in range(2):
                    isl = slice(ihalf * 1024, (ihalf + 1) * 1024)
                    pt = big_psum.tile([128, 1024], F32, name=f"qp_{b}_{cc}_{ihalf}", tag="big")
                    for kt in range(KT):
                        lhs = wqkv_sb[:, kt, cc * 128:(cc + 1) * 128]
                        for it2 in range(2):
                            s2 = slice(it2 * 512, (it2 + 1) * 512)
                            i2 = slice(ihalf * 1024 + it2 * 512, ihalf * 1024 + (it2 + 1) * 512)
                            bi = nc.tensor.matmul(
                                pt[:, s2], lhs, xts[kt][:, i2],
                                start=(kt == 0), stop=(kt == KT - 1),
                            )
                            if it2 == 1:
                                bi.ins.ldweights = False
                    rstd_bc, mrs_bc = bcs[ihalf]
                    # evict psum immediately (no LN-stats dependency), then
                    # apply the folded-LN affine on SBUF at 2x rate
                    tmp = tmp_pool.tile([128, 1024], F32, name=f"tmp_{b}_{cc}_{ihalf}", tag="tmp")
                    nc.vector.tensor_copy(tmp, pt)
                    nc.vector.tensor_mul(tmp, tmp, rstd_bc)
                    nc.vector.scalar_tensor_tensor(
                        out=sb_dst[cc][:, isl], in0=mrs_bc,
                        scalar=nwsum_sb[:, cc:cc + 1], in1=tmp,
                        op0=mybir.AluOpType.mult, op1=mybir.AluOpType.add,
                    )
                    nc.vector.tensor_scalar_add(
                        sb_dst[cc][:, isl], sb_dst[cc][:, isl], qkvb_sb[:, cc:cc + 1]
                    )

            # ---- v natural (+ ones column) via PE transpose ----
            vb = []
            for hh in range(HL):
                vn = vn_pool.tile([128, JC, DH + 1], BF16, name=f"vn_{b}_{hh}", tag=f"vn{b}{hh}")
                nc.gpsimd.memset(vn[:, :, DH:DH + 1], 1.0)
                vb.append(vn)
            vns.append(vb)
            for jc in range(JC):
                trp = big_psum.tile([128, 128], BF16, name=f"tr_{b}_{jc}", tag="big")
                nc.tensor.transpose(trp, vT[:, jc * 128:(jc + 1) * 128], ident)
                for hh in range(HL):
                    nc.vector.tensor_copy(
                        vb[hh][:, jc, 0:DH], trp[:, hh * DH:(hh + 1) * DH]
                    )

            ao_sb = ao_pool.tile([128, N], BF16, name=f"ao_{b}", tag=f"ao{b}")
            aos.append(ao_sb)

        # ---- attention: batch outer; ONE psum accumulator per group so two
        #      score tiles are in flight (PE never waits on ScalarE exp) ----
        scr3 = dscr_pool.tile([16, 1024], F32, name="scr3", tag="scr3")
        scr4 = dscr_pool.tile([16, 1024], F32, name="scr4", tag="scr4")
        for b in range(B):
            for hh in range(HL):
                hsl = slice(hh * DH, (hh + 1) * DH)
                for ihalf in range(2):
                    isl = slice(ihalf * 1024, (ihalf + 1) * 1024)
                    aop = ao_psum.tile([DH + 1, 1024], F32, name=f"aop_{b}_{hh}_{ihalf}", tag="aop")
                    for jc in range(JC):
                        jsl = slice(jc * 128, (jc + 1) * 128)
                        al_t = al_pool.tile([128, 1024], BF16, name=f"al_{b}_{hh}_{ihalf}_{jc}", tag="al")
                        nc.sync.dma_start(out=al_t, in_=al_d[hh, jsl, isl])
                        sc = big_psum.tile([128, 1024], F32, name=f"sc_{b}_{hh}_{ihalf}_{jc}", tag="big")
                        for it2 in range(2):
                            s2 = slice(it2 * 512, (it2 + 1) * 512)
                            i2 = slice(ihalf * 1024 + it2 * 512, ihalf * 1024 + (it2 + 1) * 512)
                            bi = nc.tensor.matmul(
                                sc[:, s2], kTs[b][hsl, jsl], qTs[b][hsl, i2],
                                start=True, stop=(it2 == 1),
                            )
                            if it2 == 1:
                                bi.ins.ldweights = False
                        nc.tensor.matmul(
                            sc[:, 0:512], ident, al_t[:, 0:512],
                            start=False, stop=True,
                        )
                        nc.vector.tensor_add(sc[:, 512:1024], sc[:, 512:1024], al_t[:, 512:1024])
                        at_t = at_pool.tile([128, 1024], BF16, name=f"at_{b}_{hh}_{ihalf}_{jc}", tag="at")
                        nc.scalar.activation(at_t, sc, mybir.ActivationFunctionType.Exp)
                        for it2 in range(2):
                            s2 = slice(it2 * 512, (it2 + 1) * 512)
                            bi = nc.tensor.matmul(
                                aop[:, s2], vns[b][hh][:, jc, :], at_t[:, s2],
                                start=(jc == 0), stop=(jc == JC - 1),
                            )
                            if it2 == 1:
                                bi.ins.ldweights = False
                    # evict raw attn output immediately (frees PSUM); the
                    # reciprocal runs at [128,8] via DMA reshapes and the
                    # normalize happens off the critical path
                    r = (hh * 2 + ihalf) * 2 + b
                    ao_raw = aor_pool.tile([DH + 1, 1024], F32, name=f"aor_{r}", tag="aor")
                    nc.vector.tensor_copy(ao_raw, aop)
                    nc.sync.dma_start(out=scr3[r:r + 1, :], in_=ao_raw[DH:DH + 1, :])
                    r128 = sm_pool.tile([128, 8], F32, name=f"r128_{r}", tag="r128", bufs=3)
                    nc.sync.dma_start(out=r128, in_=scr3[r:r + 1, :])
                    nc.vector.reciprocal(r128, r128)
                    nc.sync.dma_start(out=scr4[r:r + 1, :], in_=r128)
                    rr_bc = rrbc_pool.tile([DH, 1024], F32, name=f"rrbc_{r}", tag="rrbc")
                    nc.gpsimd.dma_start(
                        out=rr_bc, in_=scr4[r:r + 1, :].partition_broadcast(DH)
                    )
                    nc.vector.tensor_mul(aos[b][hsl, isl], ao_raw[0:DH, :], rr_bc)

        # ---- out projection (partial, transposed, bf16) ----
        for ihalf in range(2):
            for b in range(B):
                for ec in range(8):
                    lhs = wout_sb[:, ec * 128:(ec + 1) * 128]
                    isl = slice(ihalf * 1024, (ihalf + 1) * 1024)
                    opp = big_psum.tile([128, 1024], F32, name=f"op_{b}_{ec}_{ihalf}", tag="big")
                    for it2 in range(2):
                        s2 = slice(it2 * 512, (it2 + 1) * 512)
                        i2 = slice(ihalf * 1024 + it2 * 512, ihalf * 1024 + (it2 + 1) * 512)
                        bi = nc.tensor.matmul(opp[:, s2], lhs, aos[b][:, i2], start=True, stop=True)
                        if it2 == 1:
                            bi.ins.ldweights = False
                    ob = ob_pool.tile([128, 1024], BF16, name=f"ob_{b}_{ec}_{ihalf}", tag="ob")
                    nc.vector.tensor_copy(ob, opp)
                    nc.sync.dma_start(out=out_d[b, ec * 128:(ec + 1) * 128, isl], in_=ob)
    nc.compile()
    return nc


def make_in_maps(x, alibi_bias, ln_gamma, ln_beta, w_qkv, w_out):
    """Host-side sharding / layout prep. Returns list of 8 per-core input dicts."""
    x = np.asarray(x, np.float32)
    alibi_bias = np.asarray(alibi_bias, np.float32)
    ln_gamma = np.asarray(ln_gamma, np.float32)
    ln_beta = np.asarray(ln_beta, np.float32)
    w_qkv = np.asarray(w_qkv, np.float32)
    w_out = np.asarray(w_out, np.float32)
    BF = ml_dtypes.bfloat16

    xt = np.ascontiguousarray(x.transpose(0, 2, 1)).astype(BF)  # [B, D, N]
    # fold ln_gamma into w_qkv rows; fold attention scale into the q columns
    w_eff = w_qkv * ln_gamma[:, None]
    qkvb_full = ln_beta @ w_qkv  # [3*H*DH]
    in_maps = []
    for c in range(NCORES):
        csl = slice(c * CL, (c + 1) * CL)
        wq = w_eff[:, 0:H * DH][:, csl] * SCALE
        wk = w_eff[:, H * DH:2 * H * DH][:, csl]
        wv = w_eff[:, 2 * H * DH:3 * H * DH][:, csl]
        wqkv_c = np.ascontiguousarray(np.concatenate([wq, wk, wv], axis=1)).astype(BF)
        nwsum_c = -wqkv_c.astype(np.float64).sum(axis=0).astype(np.float32)
        qb = qkvb_full.reshape(3, H * DH)[:, csl].copy()
        qb[0] *= SCALE
        qkvb_c = np.ascontiguousarray(qb.reshape(-1))
        al_c = np.ascontiguousarray(
            alibi_bias[0, c * HL:(c + 1) * HL].transpose(0, 2, 1)
        ).astype(BF)
        wout_c = np.ascontiguousarray(w_out[csl, :]).astype(BF)
        in_maps.append({
            "xt": xt,
            "alibi": al_c,
            "wqkv": wqkv_c,
            "qkvb": qkvb_c,
            "nwsum": nwsum_c,
            "wout": wout_c,
            "ones": np.ones((128, 1), BF),
        })
    return in_maps


def kernel(x, alibi_bias, mask, ln_gamma, ln_beta, w_qkv, w_out, _trace=False):
    global _CACHED_NC
    mask = np.asarray(mask)
    assert mask.all(), "kernel assumes an all-True mask"
    if _CACHED_NC is None:
        _CACHED_NC = build_nc()
    nc = _CACHED_NC
    in_maps = make_in_maps(x, alibi_bias, ln_gamma, ln_beta, w_qkv, w_out)
    res = run_bass_kernel_spmd(nc, in_maps, core_ids=list(range(NCORES)), trace=_trace)
    out_t = np.zeros((B, D, N), np.float32)
    for c in range(NCORES):
        out_t += res.results[c]["out"].astype(np.float32)
    out = np.ascontiguousarray(out_t.transpose(0, 2, 1))
    if _trace:
        return out, res
    return out


# revision 56
# speedup vs baseline: 1.3153x; 1.0550x over previous
"""Trainium2 Bass kernel for nn_Attention (LN -> QKV -> alibi attention -> out-proj).

Full shapes: x[2,2048,1024], alibi[1,16,2048,2048], w_qkv[1024,3072], w_out[1024,1024].
Sharding: tensor-parallel over heads. Core c owns heads {2c, 2c+1} for BOTH batches.
Each core computes a partial out-projection; the host sums the 8 partials (the
tensor-parallel reduction) and transposes back.

v2 design (all matmuls bf16 -- fp32r streams at 2 cyc/row on silicon, bf16 at 1):
  - x passed host-transposed + bf16: xT[b] = [d=1024, i=2048].
  - LN folded into the QKV eviction: qkv = rstd*(W^T x) + (mean*rstd)*(-colsum(W))
    (+ beta@W). LN stats (sum, sum-sq) via matmul-with-ones run concurrently with
    the QKV matmuls on raw x; no xn materialization, no LN->QKV serialization.
  - q/k evicted bf16 (2 heads on partitions); v bf16, PE-transposed to v-natural
    [j, 64d + ones-col]; the ones column makes attn@v also emit softmax denoms.
  - attention loops h outer, batch inner: each alibi^T tile (bf16, host-transposed)
    is DMA'd once and used by both batches (16MB/core alibi traffic, the minimum).
  - scores S^T = kT_chunk^T @ qT (K=64) + identity-matmul alibi accumulate; exp on
    ScalarE (PSUM f32 -> SBUF bf16), no max-subtraction (|scores| <~ 15).
  - PSUM: one shared [128,1024] pool (bufs=2) for stats/qkv/transpose/scores/
    out-proj + one [65,1024] pool (bufs=2) holding both batches' attn accumulators.
  - out partials written bf16 transposed [b, e, i]; host sums in f32.
"""

import sys

sys.path.insert(0, "/opt/trn_rl_repo")

from contextlib import ExitStack

import numpy as np
import ml_dtypes

import concourse.bass as bass
from concourse import bacc
import concourse.mybir as mybir
import concourse.tile as tile
from concourse.bass_utils import run_bass_kernel_spmd
from concourse.masks import make_identity

F32 = mybir.dt.float32
BF16 = mybir.dt.bfloat16

B, N, D = 2, 2048, 1024
H, DH = 16, 64
NCORES = 8
HL = H // NCORES          # local heads per core = 2
CL = HL * DH              # local head channels = 128
LN_EPS = 1e-5
SCALE = DH ** -0.5
KT = D // 128             # 8 d-tiles
JC = N // 128             # 16 j-chunks
IT = N // 512             # 4 i-tiles of 512

_CACHED_NC = None


def build_nc() -> bass.Bass:
    nc = bacc.Bacc(None)
    xt_d = nc.declare_dram_parameter("xt", [B, D, N], BF16, isOutput=False)
    al_d = nc.declare_dram_parameter("alibi", [HL, N, N], BF16, isOutput=False)
    wqkv_d = nc.declare_dram_parameter("wqkv", [D, 3 * CL], BF16, isOutput=False)
    qkvb_d = nc.declare_dram_parameter("qkvb", [3 * CL], F32, isOutput=False)
    nwsum_d = nc.declare_dram_parameter("nwsum", [3 * CL], F32, isOutput=False)
    wout_d = nc.declare_dram_parameter("wout", [CL, D], BF16, isOutput=False)
    ones_d = nc.declare_dram_parameter("ones", [128, 1], BF16, isOutput=False)
    out_d = nc.declare_dram_parameter("out", [B, D, N], BF16, isOutput=True)

    with tile.TileContext(nc) as tc, ExitStack() as ctx:
        ep = lambda **kw: ctx.enter_context(tc.tile_pool(**kw))
        cpool = ep(name="const", bufs=1)
        xt_pool = ep(name="xt", bufs=11)
        sq_pool = ep(name="sq", bufs=4)
        sm_pool = ep(name="small", bufs=2)
        tmp_pool = ep(name="tmp", bufs=4)
        qk_pool = ep(name="qk", bufs=1)      # per-batch tiles, all resident
        vt_pool = ep(name="vt", bufs=2)
        vn_pool = ep(name="vn", bufs=1)      # 4 resident tiles (b x head)
        al_pool = ep(name="al", bufs=8)
        at_pool = ep(name="at", bufs=8)
        ao_pool = ep(name="aos", bufs=1)
        ob_pool = ep(name="ob", bufs=4)
        bc_pool = ep(name="bc", bufs=4)
        rrbc_pool = ep(name="rrbc", bufs=3)
        aor_pool = ep(name="aor", bufs=3)
        dscr_pool = ep(name="dscr", bufs=2, space="DRAM")
        big_psum = ep(name="ps_big", bufs=3, space="PSUM")
        ao_psum = ep(name="ps_ao", bufs=1, space="PSUM")

        # ---- constants ----
        ident = cpool.tile([128, 128], BF16, name="ident")
        make_identity(nc, ident)
        zero_sb = cpool.tile([128, 1], F32, name="zero_sb")
        nc.vector.memset(zero_sb, 0.0)
        nc.const_aps.aps[(F32, 0.0)] = zero_sb[:, 0:1]
        eps_sb = cpool.tile([128, 1], F32, name="eps_sb")
        nc.vector.memset(eps_sb, LN_EPS)
        ones_sb = cpool.tile([128, 1], BF16, name="ones_sb")
        nc.sync.dma_start(out=ones_sb, in_=ones_d[:, :])
        wqkv_sb = cpool.tile([128, KT, 3 * CL], BF16, name="wqkv_sb")
        nc.sync.dma_start(out=wqkv_sb, in_=wqkv_d.rearrange("(t p) c -> p t c", p=128))
        qkvb_sb = cpool.tile([128, 3], F32, name="qkvb_sb")
        nc.sync.dma_start(out=qkvb_sb, in_=qkvb_d.rearrange("(c p) -> p c", p=128))
        nwsum_sb = cpool.tile([128, 3], F32, name="nwsum_sb")
        nc.sync.dma_start(out=nwsum_sb, in_=nwsum_d.rearrange("(c p) -> p c", p=128))
        wout_sb = cpool.tile([128, D], BF16, name="wout_sb")
        nc.sync.dma_start(out=wout_sb, in_=wout_d[:, :])

        qTs, kTs, vns, aos = [], [], [], []
        for b in range(B):
            # ---- load xT (bf16) ----
            xts = []
            for kt in range(KT):
                xt_t = xt_pool.tile([128, N], BF16, name=f"xt_{b}_{kt}", tag="xt")
                nc.sync.dma_start(out=xt_t, in_=xt_d[b, kt * 128:(kt + 1) * 128, :])
                xts.append(xt_t)

            # ---- LN stats (sum, sumsq) via matmul-with-ones ----

            scr = dscr_pool.tile([2, N], F32, name=f"scr_{b}", tag="scr")
            st = sm_pool.tile([128, 112], F32, name=f"st_{b}", tag="st128")
            for ihalf in range(2):
                isl = slice(ihalf * 1024, (ihalf + 1) * 1024)
                rows = sm_pool.tile([1, N], F32, name=f"rows_{b}_{ihalf}", tag="rows", bufs=1)
                sum_ps = big_psum.tile([1, 1024], F32, name=f"sum_{b}_{ihalf}", tag="big")
                sq_ps = big_psum.tile([33, 1024], F32, name=f"ssq_{b}_{ihalf}", tag="big")
                for kt in range(KT):
                    xsq = sq_pool.tile([128, 1024], BF16, name=f"xsq_{b}_{ihalf}_{kt}", tag="sq")
                    nc.gpsimd.tensor_mul(xsq, xts[kt][:, isl], xts[kt][:, isl])
                    for it2 in range(2):
                        s2 = slice(it2 * 512, (it2 + 1) * 512)
                        i2 = slice(ihalf * 1024 + it2 * 512, ihalf * 1024 + (it2 + 1) * 512)
                        nc.tensor.matmul(
                            sum_ps[0:1, s2], ones_sb, xts[kt][:, i2],
                            start=(kt == 0), stop=(kt == KT - 1),
                        )
                        nc.tensor.matmul(
                            sq_ps[32:33, s2], ones_sb, xsq[:, s2],
                            start=(kt == 0), stop=(kt == KT - 1),
                            tile_position=(0, 32),
                        )
                nc.vector.tensor_copy(rows[0:1, 0:1024], sum_ps)
                nc.vector.tensor_copy(rows[0:1, 1024:2048], sq_ps[32:33, :])
                nc.sync.dma_start(out=st[:, ihalf * 8:(ihalf + 1) * 8], in_=rows[0:1, 0:1024])
                nc.sync.dma_start(out=st[:, 16 + ihalf * 8:16 + (ihalf + 1) * 8], in_=rows[0:1, 1024:2048])
            # stat cols: 0:16 sum128, 16:32 sumsq128, 32:48 mean, 48:64 ex2,
            # 64:80 -var, 80:96 rstd, 96:112 mean*rstd
            mean, ex2 = st[:, 32:48], st[:, 48:64]
            nvar, rstd, mrs = st[:, 64:80], st[:, 80:96], st[:, 96:112]
            nc.vector.tensor_scalar_mul(mean, st[:, 0:16], 1.0 / D)
            nc.vector.tensor_scalar_mul(ex2, st[:, 16:32], 1.0 / D)
            nc.vector.tensor_mul(nvar, mean, mean)
            nc.vector.tensor_sub(nvar, nvar, ex2)  # mean^2 - E[x^2] = -var
            nc.scalar.activation(
                rstd, nvar, mybir.ActivationFunctionType.Sqrt,
                bias=eps_sb[:, 0:1], scale=-1.0,
            )  # sqrt(var + eps)
            nc.vector.reciprocal(rstd, rstd)
            nc.vector.tensor_mul(mrs, mean, rstd)
            for ihalf in range(2):
                c8 = slice(ihalf * 8, (ihalf + 1) * 8)
                isl = slice(ihalf * 1024, (ihalf + 1) * 1024)
                nc.sync.dma_start(out=scr[0:1, isl], in_=rstd[:, c8])
                nc.sync.dma_start(out=scr[1:2, isl], in_=mrs[:, c8])
            # broadcast rows (DRAM -> 128 partitions)
            bcs = []
            for ihalf in range(2):
                isl = slice(ihalf * 1024, (ihalf + 1) * 1024)
                rstd_bc = bc_pool.tile([128, 1024], F32, name=f"rsbc_{b}_{ihalf}", tag="bc")
                nc.gpsimd.dma_start(out=rstd_bc, in_=scr[0:1, isl].partition_broadcast(128))
                mrs_bc = bc_pool.tile([128, 1024], F32, name=f"mrbc_{b}_{ihalf}", tag="bc")
                nc.gpsimd.dma_start(out=mrs_bc, in_=scr[1:2, isl].partition_broadcast(128))
                bcs.append((rstd_bc, mrs_bc))

            # ---- QKV projection on raw x; LN affine applied at eviction ----
            qT = qk_pool.tile([128, N], BF16, name=f"qT_{b}", tag=f"qT{b}")
            kT = qk_pool.tile([128, N], BF16, name=f"kT_{b}", tag=f"kT{b}")
            vT = vt_pool.tile([128, N], BF16, name=f"vT_{b}", tag="vT")
            qTs.append(qT)
            kTs.append(kT)
            sb_dst = [qT, kT, vT]
            for cc in (1, 2, 0):
                for ihalf in range(2):
                    isl = slice(ihalf * 1024, (ihalf + 1) * 1024)
                    pt = big_psum.tile([128, 1024], F32, name=f"qp_{b}_{cc}_{ihalf}", tag="big")
                    for kt in range(KT):
                        lhs = wqkv_sb[:, kt, cc * 128:(cc + 1) * 128]
                        for it2 in range(2):
                            s2 = slice(it2 * 512, (it2 + 1) * 512)
                            i2 = slice(ihalf * 1024 + it2 * 512, ihalf * 1024 + (it2 + 1) * 512)
                            bi = nc.tensor.matmul(
                                pt[:, s2], lhs, xts[kt][:, i2],
                                start=(kt == 0), stop=(kt == KT - 1),
                            )
                            if it2 == 1:
                                bi.ins.ldweights = False
                    rstd_bc, mrs_bc = bcs[ihalf]
                    tmp = tmp_pool.tile([128, 1024], F32, name=f"tmp_{b}_{cc}_{ihalf}", tag="tmp")
                    nc.vector.tensor_mul(tmp, pt, rstd_bc)
                    nc.vector.scalar_tensor_tensor(
                        out=sb_dst[cc][:, isl], in0=mrs_bc,
                        scalar=nwsum_sb[:, cc:cc + 1], in1=tmp,
                        op0=mybir.AluOpType.mult, op1=mybir.AluOpType.add,
                    )
                    nc.vector.tensor_scalar_add(
                        sb_dst[cc][:, isl], sb_dst[cc][:, isl], qkvb_sb[:, cc:cc + 1]
                    )

            # ---- v natural (+ ones column) via PE transpose ----
            vb = []
            for hh in range(HL):
                vn = vn_pool.tile([128, JC, DH + 1], BF16, name=f"vn_{b}_{hh}", tag=f"vn{b}{hh}")
                nc.gpsimd.memset(vn[:, :, DH:DH + 1], 1.0)
                vb.append(vn)
            vns.append(vb)
            for jc in range(JC):
                trp = big_psum.tile([128, 128], BF16, name=f"tr_{b}_{jc}", tag="big")
                nc.tensor.transpose(trp, vT[:, jc * 128:(jc + 1) * 128], ident)
                for hh in range(HL):
                    nc.vector.tensor_copy(
                        vb[hh][:, jc, 0:DH], trp[:, hh * DH:(hh + 1) * DH]
                    )

            ao_sb = ao_pool.tile([128, N], BF16, name=f"ao_{b}", tag=f"ao{b}")
            aos.append(ao_sb)

        # ---- attention: batch outer; ONE psum accumulator per group so two
        #      score tiles are in flight (PE never waits on ScalarE exp) ----
        scr3 = dscr_pool.tile([16, 1024], F32, name="scr3", tag="scr3")
        scr4 = dscr_pool.tile([16, 1024], F32, name="scr4", tag="scr4")
        for b in range(B):
            for hh in range(HL):
                hsl = slice(hh * DH, (hh + 1) * DH)
                for ihalf in range(2):
                    isl = slice(ihalf * 1024, (ihalf + 1) * 1024)
                    aop = ao_psum.tile([DH + 1, 1024], F32, name=f"aop_{b}_{hh}_{ihalf}", tag="aop")
                    for jc in range(JC):
                        jsl = slice(jc * 128, (jc + 1) * 128)
                        al_t = al_pool.tile([128, 1024], BF16, name=f"al_{b}_{hh}_{ihalf}_{jc}", tag="al")
                        nc.sync.dma_start(out=al_t, in_=al_d[hh, jsl, isl])
                        sc = big_psum.tile([128, 1024], F32, name=f"sc_{b}_{hh}_{ihalf}_{jc}", tag="big")
                        for it2 in range(2):
                            s2 = slice(it2 * 512, (it2 + 1) * 512)
                            i2 = slice(ihalf * 1024 + it2 * 512, ihalf * 1024 + (it2 + 1) * 512)
                            bi = nc.tensor.matmul(
                                sc[:, s2], kTs[b][hsl, jsl], qTs[b][hsl, i2],
                                start=True, stop=(it2 == 1),
                            )
                            if it2 == 1:
                                bi.ins.ldweights = False
                        nc.tensor.matmul(
                            sc[:, 0:512], ident, al_t[:, 0:512],
                            start=False, stop=True,
                        )
                        nc.vector.tensor_add(sc[:, 512:1024], sc[:, 512:1024], al_t[:, 512:1024])
                        at_t = at_pool.tile([128, 1024], BF16, name=f"at_{b}_{hh}_{ihalf}_{jc}", tag="at")
                        nc.scalar.activation(at_t, sc, mybir.ActivationFunctionType.Exp)
                        for it2 in range(2):
                            s2 = slice(it2 * 512, (it2 + 1) * 512)
                            bi = nc.tensor.matmul(
                                aop[:, s2], vns[b][hh][:, jc, :], at_t[:, s2],
                                start=(jc == 0), stop=(jc == JC - 1),
                            )
                            if it2 == 1:
                                bi.ins.ldweights = False
                    # evict raw attn output immediately (frees PSUM); the
                    # reciprocal runs at [128,8] via DMA reshapes and the
                    # normalize happens off the critical path
                    r = (hh * 2 + ihalf) * 2 + b
                    ao_raw = aor_pool.tile([DH + 1, 1024], F32, name=f"aor_{r}", tag="aor")
                    nc.vector.tensor_copy(ao_raw, aop)
                    nc.sync.dma_start(out=scr3[r:r + 1, :], in_=ao_raw[DH:DH + 1, :])
                    r128 = sm_pool.tile([128, 8], F32, name=f"r128_{r}", tag="r128", bufs=3)
                    nc.sync.dma_start(out=r128, in_=scr3[r:r + 1, :])
                    nc.vector.reciprocal(r128, r128)
                    nc.sync.dma_start(out=scr4[r:r + 1, :], in_=r128)
                    rr_bc = rrbc_pool.tile([DH, 1024], F32, name=f"rrbc_{r}", tag="rrbc")
                    nc.gpsimd.dma_start(
                        out=rr_bc, in_=scr4[r:r + 1, :].partition_broadcast(DH)
                    )
                    nc.vector.tensor_mul(aos[b][hsl, isl], ao_raw[0:DH, :], rr_bc)

        # ---- out projection (partial, transposed, bf16) ----
        for ihalf in range(2):
            for b in range(B):
                for ec in range(8):
                    lhs = wout_sb[:, ec * 128:(ec + 1) * 128]
                    isl = slice(ihalf * 1024, (ihalf + 1) * 1024)
                    opp = big_psum.tile([128, 1024], F32, name=f"op_{b}_{ec}_{ihalf}", tag="big")
                    for it2 in range(2):
                        s2 = slice(it2 * 512, (it2 + 1) * 512)
                        i2 = slice(ihalf * 1024 + it2 * 512, ihalf * 1024 + (it2 + 1) * 512)
                        bi = nc.tensor.matmul(opp[:, s2], lhs, aos[b][:, i2], start=True, stop=True)
                        if it2 == 1:
                            bi.ins.ldweights = False
                    ob = ob_pool.tile([128, 1024], BF16, name=f"ob_{b}_{ec}_{ihalf}", tag="ob")
                    nc.vector.tensor_copy(ob, opp)
                    nc.sync.dma_start(out=out_d[b, ec * 128:(ec + 1) * 128, isl], in_=ob)
    nc.compile()
    return nc


def make_in_maps(x, alibi_bias, ln_gamma, ln_beta, w_qkv, w_out):
    """Host-side sharding / layout prep. Returns list of 8 per-core input dicts."""
    x = np.asarray(x, np.float32)
    alibi_bias = np.asarray(alibi_bias, np.float32)
    ln_gamma = np.asarray(ln_gamma, np.float32)
    ln_beta = np.asarray(ln_beta, np.float32)
    w_qkv = np.asarray(w_qkv, np.float32)
    w_out = np.asarray(w_out, np.float32)
    BF = ml_dtypes.bfloat16

    xt = np.ascontiguousarray(x.transpose(0, 2, 1)).astype(BF)  # [B, D, N]
    # fold ln_gamma into w_qkv rows; fold attention scale into the q columns
    w_eff = w_qkv * ln_gamma[:, None]
    qkvb_full = ln_beta @ w_qkv  # [3*H*DH]
    in_maps = []
    for c in range(NCORES):
        csl = slice(c * CL, (c + 1) * CL)
        wq = w_eff[:, 0:H * DH][:, csl] * SCALE
        wk = w_eff[:, H * DH:2 * H * DH][:, csl]
        wv = w_eff[:, 2 * H * DH:3 * H * DH][:, csl]
        wqkv_c = np.ascontiguousarray(np.concatenate([wq, wk, wv], axis=1)).astype(BF)
        nwsum_c = -wqkv_c.astype(np.float64).sum(axis=0).astype(np.float32)
        qb = qkvb_full.reshape(3, H * DH)[:, csl].copy()
        qb[0] *= SCALE
        qkvb_c = np.ascontiguousarray(qb.reshape(-1))
        al_c = np.ascontiguousarray(
            alibi_bias[0, c * HL:(c + 1) * HL].transpose(0, 2, 1)
        ).astype(BF)
        wout_c = np.ascontiguousarray(w_out[csl, :]).astype(BF)
        in_maps.append({
            "xt": xt,
            "alibi": al_c,
            "wqkv": wqkv_c,
            "qkvb": qkvb_c,
            "nwsum": nwsum_c,
            "wout": wout_c,
            "ones": np.ones((128, 1), BF),
        })
    return in_maps


def kernel(x, alibi_bias, mask, ln_gamma, ln_beta, w_qkv, w_out, _trace=False):
    global _CACHED_NC
    mask = np.asarray(mask)
    assert mask.all(), "kernel assumes an all-True mask"
    if _CACHED_NC is None:
        _CACHED_NC = build_nc()
    nc = _CACHED_NC
    in_maps = make_in_maps(x, alibi_bias, ln_gamma, ln_beta, w_qkv, w_out)
    res = run_bass_kernel_spmd(nc, in_maps, core_ids=list(range(NCORES)), trace=_trace)
    out_t = np.zeros((B, D, N), np.float32)
    for c in range(NCORES):
        out_t += res.results[c]["out"].astype(np.float32)
    out = np.ascontiguousarray(out_t.transpose(0, 2, 1))
    if _trace:
        return out, res
    return out


# revision 57
# speedup vs baseline: 1.3756x; 1.0458x over previous
"""Trainium2 Bass kernel for nn_Attention (LN -> QKV -> alibi attention -> out-proj).

Full shapes: x[2,2048,1024], alibi[1,16,2048,2048], w_qkv[1024,3072], w_out[1024,1024].
Sharding: tensor-parallel over heads. Core c owns heads {2c, 2c+1} for BOTH batches.
Each core computes a partial out-projection; the host sums the 8 partials (the
tensor-parallel reduction) and transposes back.

v2 design (all matmuls bf16 -- fp32r streams at 2 cyc/row on silicon, bf16 at 1):
  - x passed host-transposed + bf16: xT[b] = [d=1024, i=2048].
  - LN folded into the QKV eviction: qkv = rstd*(W^T x) + (mean*rstd)*(-colsum(W))
    (+ beta@W). LN stats (sum, sum-sq) via matmul-with-ones run concurrently with
    the QKV matmuls on raw x; no xn materialization, no LN->QKV serialization.
  - q/k evicted bf16 (2 heads on partitions); v bf16, PE-transposed to v-natural
    [j, 64d + ones-col]; the ones column makes attn@v also emit softmax denoms.
  - attention loops h outer, batch inner: each alibi^T tile (bf16, host-transposed)
    is DMA'd once and used by both batches (16MB/core alibi traffic, the minimum).
  - scores S^T = kT_chunk^T @ qT (K=64) + identity-matmul alibi accumulate; exp on
    ScalarE (PSUM f32 -> SBUF bf16), no max-subtraction (|scores| <~ 15).
  - PSUM: one shared [128,1024] pool (bufs=2) for stats/qkv/transpose/scores/
    out-proj + one [65,1024] pool (bufs=2) holding both batches' attn accumulators.
  - out partials written bf16 transposed [b, e, i]; host sums in f32.
"""

import sys

sys.path.insert(0, "/opt/trn_rl_repo")

from contextlib import ExitStack

import numpy as np
import ml_dtypes

import concourse.bass as bass
from concourse import bacc
import concourse.mybir as mybir
import concourse.tile as tile
from concourse.bass_utils import run_bass_kernel_spmd
from concourse.masks import make_identity

F32 = mybir.dt.float32
BF16 = mybir.dt.bfloat16

B, N, D = 2, 2048, 1024
H, DH = 16, 64
NCORES = 8
HL = H // NCORES          # local heads per core = 2
CL = HL * DH              # local head channels = 128
LN_EPS = 1e-5
SCALE = DH ** -0.5
KT = D // 128             # 8 d-tiles
JC = N // 128             # 16 j-chunks
IT = N // 512             # 4 i-tiles of 512

_CACHED_NC = None


def build_nc() -> bass.Bass:
    nc = bacc.Bacc(None)
    xt_d = nc.declare_dram_parameter("xt", [B, D, N], BF16, isOutput=False)
    al_d = nc.declare_dram_parameter("alibi", [HL, N, N], BF16, isOutput=False)
    wqkv_d = nc.declare_dram_parameter("wqkv", [D, 3 * CL], BF16, isOutput=False)
    qkvb_d = nc.declare_dram_parameter("qkvb", [3 * CL], F32, isOutput=False)
    nwsum_d = nc.declare_dram_parameter("nwsum", [3 * CL], F32, isOutput=False)
    wout_d = nc.declare_dram_parameter("wout", [CL, D], BF16, isOutput=False)
    ones_d = nc.declare_dram_parameter("ones", [128, 1], BF16, isOutput=False)
    out_d = nc.declare_dram_parameter("out", [B, D, N], BF16, isOutput=True)

    with tile.TileContext(nc) as tc, ExitStack() as ctx:
        ep = lambda **kw: ctx.enter_context(tc.tile_pool(**kw))
        cpool = ep(name="const", bufs=1)
        xt_pool = ep(name="xt", bufs=11)
        sq_pool = ep(name="sq", bufs=4)
        sm_pool = ep(name="small", bufs=2)
        tmp_pool = ep(name="tmp", bufs=4)
        qk_pool = ep(name="qk", bufs=1)      # per-batch tiles, all resident
        vt_pool = ep(name="vt", bufs=2)
        vn_pool = ep(name="vn", bufs=1)      # 4 resident tiles (b x head)
        al_pool = ep(name="al", bufs=8)
        at_pool = ep(name="at", bufs=8)
        ao_pool = ep(name="aos", bufs=1)
        ob_pool = ep(name="ob", bufs=4)
        bc_pool = ep(name="bc", bufs=4)
        rrbc_pool = ep(name="rrbc", bufs=3)
        aor_pool = ep(name="aor", bufs=3)
        dscr_pool = ep(name="dscr", bufs=2, space="DRAM")
        big_psum = ep(name="ps_big", bufs=3, space="PSUM")
        ao_psum = ep(name="ps_ao", bufs=1, space="PSUM")

        # ---- constants ----
        ident = cpool.tile([128, 128], BF16, name="ident")
        make_identity(nc, ident)
        zero_sb = cpool.tile([128, 1], F32, name="zero_sb")
        nc.vector.memset(zero_sb, 0.0)
        nc.const_aps.aps[(F32, 0.0)] = zero_sb[:, 0:1]
        eps_sb = cpool.tile([128, 1], F32, name="eps_sb")
        nc.vector.memset(eps_sb, LN_EPS)
        ones_sb = cpool.tile([128, 1], BF16, name="ones_sb")
        nc.sync.dma_start(out=ones_sb, in_=ones_d[:, :])
        wqkv_sb = cpool.tile([128, KT, 3 * CL], BF16, name="wqkv_sb")
        nc.sync.dma_start(out=wqkv_sb, in_=wqkv_d.rearrange("(t p) c -> p t c", p=128))
        qkvb_sb = cpool.tile([128, 3], F32, name="qkvb_sb")
        nc.sync.dma_start(out=qkvb_sb, in_=qkvb_d.rearrange("(c p) -> p c", p=128))
        nwsum_sb = cpool.tile([128, 3], F32, name="nwsum_sb")
        nc.sync.dma_start(out=nwsum_sb, in_=nwsum_d.rearrange("(c p) -> p c", p=128))
        wout_sb = cpool.tile([128, D], BF16, name="wout_sb")
        nc.sync.dma_start(out=wout_sb, in_=wout_d[:, :])

        qTs, kTs, vns, aos = [], [], [], []
        for b in range(B):
            # ---- load xT (bf16) ----
            xts = []
            for kt in range(KT):
                xt_t = xt_pool.tile([128, N], BF16, name=f"xt_{b}_{kt}", tag="xt")
                nc.sync.dma_start(out=xt_t, in_=xt_d[b, kt * 128:(kt + 1) * 128, :])
                xts.append(xt_t)

            # ---- LN stats (sum, sumsq) via matmul-with-ones ----

            scr = dscr_pool.tile([2, N], F32, name=f"scr_{b}", tag="scr")
            st = sm_pool.tile([128, 112], F32, name=f"st_{b}", tag="st128")
            for ihalf in range(2):
                isl = slice(ihalf * 1024, (ihalf + 1) * 1024)
                rows = sm_pool.tile([1, N], F32, name=f"rows_{b}_{ihalf}", tag="rows", bufs=1)
                sum_ps = big_psum.tile([1, 1024], F32, name=f"sum_{b}_{ihalf}", tag="big")
                sq_ps = big_psum.tile([33, 1024], F32, name=f"ssq_{b}_{ihalf}", tag="big")
                for kt in range(KT):
                    xsq = sq_pool.tile([128, 1024], BF16, name=f"xsq_{b}_{ihalf}_{kt}", tag="sq")
                    nc.vector.tensor_mul(xsq, xts[kt][:, isl], xts[kt][:, isl])
                    for it2 in range(2):
                        s2 = slice(it2 * 512, (it2 + 1) * 512)
                        i2 = slice(ihalf * 1024 + it2 * 512, ihalf * 1024 + (it2 + 1) * 512)
                        nc.tensor.matmul(
                            sum_ps[0:1, s2], ones_sb, xts[kt][:, i2],
                            start=(kt == 0), stop=(kt == KT - 1),
                        )
                        nc.tensor.matmul(
                            sq_ps[32:33, s2], ones_sb, xsq[:, s2],
                            start=(kt == 0), stop=(kt == KT - 1),
                            tile_position=(0, 32),
                        )
                nc.vector.tensor_copy(rows[0:1, 0:1024], sum_ps)
                nc.vector.tensor_copy(rows[0:1, 1024:2048], sq_ps[32:33, :])
                nc.sync.dma_start(out=st[:, ihalf * 8:(ihalf + 1) * 8], in_=rows[0:1, 0:1024])
                nc.sync.dma_start(out=st[:, 16 + ihalf * 8:16 + (ihalf + 1) * 8], in_=rows[0:1, 1024:2048])
            # stat cols: 0:16 sum128, 16:32 sumsq128, 32:48 mean, 48:64 ex2,
            # 64:80 -var, 80:96 rstd, 96:112 mean*rstd
            mean, ex2 = st[:, 32:48], st[:, 48:64]
            nvar, rstd, mrs = st[:, 64:80], st[:, 80:96], st[:, 96:112]
            nc.vector.tensor_scalar_mul(mean, st[:, 0:16], 1.0 / D)
            nc.vector.tensor_scalar_mul(ex2, st[:, 16:32], 1.0 / D)
            nc.vector.tensor_mul(nvar, mean, mean)
            nc.vector.tensor_sub(nvar, nvar, ex2)  # mean^2 - E[x^2] = -var
            nc.scalar.activation(
                rstd, nvar, mybir.ActivationFunctionType.Sqrt,
                bias=eps_sb[:, 0:1], scale=-1.0,
            )  # sqrt(var + eps)
            nc.vector.reciprocal(rstd, rstd)
            nc.vector.tensor_mul(mrs, mean, rstd)
            for ihalf in range(2):
                c8 = slice(ihalf * 8, (ihalf + 1) * 8)
                isl = slice(ihalf * 1024, (ihalf + 1) * 1024)
                nc.sync.dma_start(out=scr[0:1, isl], in_=rstd[:, c8])
                nc.sync.dma_start(out=scr[1:2, isl], in_=mrs[:, c8])
            # broadcast rows (DRAM -> 128 partitions)
            bcs = []
            for ihalf in range(2):
                isl = slice(ihalf * 1024, (ihalf + 1) * 1024)
                rstd_bc = bc_pool.tile([128, 1024], F32, name=f"rsbc_{b}_{ihalf}", tag="bc")
                nc.gpsimd.dma_start(out=rstd_bc, in_=scr[0:1, isl].partition_broadcast(128))
                mrs_bc = bc_pool.tile([128, 1024], F32, name=f"mrbc_{b}_{ihalf}", tag="bc")
                nc.gpsimd.dma_start(out=mrs_bc, in_=scr[1:2, isl].partition_broadcast(128))
                bcs.append((rstd_bc, mrs_bc))

            # ---- QKV projection on raw x; LN affine applied at eviction ----
            qT = qk_pool.tile([128, N], BF16, name=f"qT_{b}", tag=f"qT{b}")
            kT = qk_pool.tile([128, N], BF16, name=f"kT_{b}", tag=f"kT{b}")
            vT = vt_pool.tile([128, N], BF16, name=f"vT_{b}", tag="vT")
            qTs.append(qT)
            kTs.append(kT)
            sb_dst = [qT, kT, vT]
            for cc in (1, 2, 0):
                for ihalf in range(2):
                    isl = slice(ihalf * 1024, (ihalf + 1) * 1024)
                    pt = big_psum.tile([128, 1024], F32, name=f"qp_{b}_{cc}_{ihalf}", tag="big")
                    for kt in range(KT):
                        lhs = wqkv_sb[:, kt, cc * 128:(cc + 1) * 128]
                        for it2 in range(2):
                            s2 = slice(it2 * 512, (it2 + 1) * 512)
                            i2 = slice(ihalf * 1024 + it2 * 512, ihalf * 1024 + (it2 + 1) * 512)
                            bi = nc.tensor.matmul(
                                pt[:, s2], lhs, xts[kt][:, i2],
                                start=(kt == 0), stop=(kt == KT - 1),
                            )
                            if it2 == 1:
                                bi.ins.ldweights = False
                    rstd_bc, mrs_bc = bcs[ihalf]
                    tmp = tmp_pool.tile([128, 1024], F32, name=f"tmp_{b}_{cc}_{ihalf}", tag="tmp")
                    nc.vector.tensor_mul(tmp, pt, rstd_bc)
                    nc.vector.scalar_tensor_tensor(
                        out=sb_dst[cc][:, isl], in0=mrs_bc,
                        scalar=nwsum_sb[:, cc:cc + 1], in1=tmp,
                        op0=mybir.AluOpType.mult, op1=mybir.AluOpType.add,
                    )
                    nc.vector.tensor_scalar_add(
                        sb_dst[cc][:, isl], sb_dst[cc][:, isl], qkvb_sb[:, cc:cc + 1]
                    )

            # ---- v natural (+ ones column) via PE transpose ----
            vb = []
            for hh in range(HL):
                vn = vn_pool.tile([128, JC, DH + 1], BF16, name=f"vn_{b}_{hh}", tag=f"vn{b}{hh}")
                nc.gpsimd.memset(vn[:, :, DH:DH + 1], 1.0)
                vb.append(vn)
            vns.append(vb)
            for jc in range(JC):
                trp = big_psum.tile([128, 128], BF16, name=f"tr_{b}_{jc}", tag="big")
                nc.tensor.transpose(trp, vT[:, jc * 128:(jc + 1) * 128], ident)
                for hh in range(HL):
                    nc.vector.tensor_copy(
                        vb[hh][:, jc, 0:DH], trp[:, hh * DH:(hh + 1) * DH]
                    )

            ao_sb = ao_pool.tile([128, N], BF16, name=f"ao_{b}", tag=f"ao{b}")
            aos.append(ao_sb)

        # ---- attention: batch outer; ONE psum accumulator per group so two
        #      score tiles are in flight (PE never waits on ScalarE exp) ----
        scr3 = dscr_pool.tile([16, 1024], F32, name="scr3", tag="scr3")
        scr4 = dscr_pool.tile([16, 1024], F32, name="scr4", tag="scr4")
        for b in range(B):
            for hh in range(HL):
                hsl = slice(hh * DH, (hh + 1) * DH)
                for ihalf in range(2):
                    isl = slice(ihalf * 1024, (ihalf + 1) * 1024)
                    aop = ao_psum.tile([DH + 1, 1024], F32, name=f"aop_{b}_{hh}_{ihalf}", tag="aop")
                    for jc in range(JC):
                        jsl = slice(jc * 128, (jc + 1) * 128)
                        al_t = al_pool.tile([128, 1024], BF16, name=f"al_{b}_{hh}_{ihalf}_{jc}", tag="al")
                        nc.sync.dma_start(out=al_t, in_=al_d[hh, jsl, isl])
                        sc = big_psum.tile([128, 1024], F32, name=f"sc_{b}_{hh}_{ihalf}_{jc}", tag="big")
                        for it2 in range(2):
                            s2 = slice(it2 * 512, (it2 + 1) * 512)
                            i2 = slice(ihalf * 1024 + it2 * 512, ihalf * 1024 + (it2 + 1) * 512)
                            bi = nc.tensor.matmul(
                                sc[:, s2], kTs[b][hsl, jsl], qTs[b][hsl, i2],
                                start=True, stop=(it2 == 1),
                            )
                            if it2 == 1:
                                bi.ins.ldweights = False
                        nc.tensor.matmul(
                            sc[:, 0:512], ident, al_t[:, 0:512],
                            start=False, stop=True,
                        )
                        nc.vector.tensor_add(sc[:, 512:1024], sc[:, 512:1024], al_t[:, 512:1024])
                        at_t = at_pool.tile([128, 1024], BF16, name=f"at_{b}_{hh}_{ihalf}_{jc}", tag="at")
                        nc.scalar.activation(at_t, sc, mybir.ActivationFunctionType.Exp)
                        for it2 in range(2):
                            s2 = slice(it2 * 512, (it2 + 1) * 512)
                            bi = nc.tensor.matmul(
                                aop[:, s2], vns[b][hh][:, jc, :], at_t[:, s2],
                                start=(jc == 0), stop=(jc == JC - 1),
                            )
                            if it2 == 1:
                                bi.ins.ldweights = False
                    # evict raw attn output immediately (frees PSUM); the
                    # reciprocal runs at [128,8] via DMA reshapes and the
                    # normalize happens off the critical path
                    r = (hh * 2 + ihalf) * 2 + b
                    ao_raw = aor_pool.tile([DH + 1, 1024], F32, name=f"aor_{r}", tag="aor")
                    nc.vector.tensor_copy(ao_raw, aop)
                    nc.sync.dma_start(out=scr3[r:r + 1, :], in_=ao_raw[DH:DH + 1, :])
                    r128 = sm_pool.tile([128, 8], F32, name=f"r128_{r}", tag="r128", bufs=3)
                    nc.sync.dma_start(out=r128, in_=scr3[r:r + 1, :])
                    nc.vector.reciprocal(r128, r128)
                    nc.sync.dma_start(out=scr4[r:r + 1, :], in_=r128)
                    rr_bc = rrbc_pool.tile([DH, 1024], F32, name=f"rrbc_{r}", tag="rrbc")
                    nc.gpsimd.dma_start(
                        out=rr_bc, in_=scr4[r:r + 1, :].partition_broadcast(DH)
                    )
                    nc.vector.tensor_mul(aos[b][hsl, isl], ao_raw[0:DH, :], rr_bc)

        # ---- out projection (partial, transposed, bf16) ----
        for ihalf in range(2):
            for b in range(B):
                for ec in range(8):
                    lhs = wout_sb[:, ec * 128:(ec + 1) * 128]
                    isl = slice(ihalf * 1024, (ihalf + 1) * 1024)
                    opp = big_psum.tile([128, 1024], F32, name=f"op_{b}_{ec}_{ihalf}", tag="big")
                    for it2 in range(2):
                        s2 = slice(it2 * 512, (it2 + 1) * 512)
                        i2 = slice(ihalf * 1024 + it2 * 512, ihalf * 1024 + (it2 + 1) * 512)
                        bi = nc.tensor.matmul(opp[:, s2], lhs, aos[b][:, i2], start=True, stop=True)
                        if it2 == 1:
                            bi.ins.ldweights = False
                    ob = ob_pool.tile([128, 1024], BF16, name=f"ob_{b}_{ec}_{ihalf}", tag="ob")
                    nc.vector.tensor_copy(ob, opp)
                    nc.sync.dma_start(out=out_d[b, ec * 128:(ec + 1) * 128, isl], in_=ob)
    nc.compile()
    return nc


def make_in_maps(x, alibi_bias, ln_gamma, ln_beta, w_qkv, w_out):
    """Host-side sharding / layout prep. Returns list of 8 per-core input dicts."""
    x = np.asarray(x, np.float32)
    alibi_bias = np.asarray(alibi_bias, np.float32)
    ln_gamma = np.asarray(ln_gamma, np.float32)
    ln_beta = np.asarray(ln_beta, np.float32)
    w_qkv = np.asarray(w_qkv, np.float32)
    w_out = np.asarray(w_out, np.float32)
    BF = ml_dtypes.bfloat16

    xt = np.ascontiguousarray(x.transpose(0, 2, 1)).astype(BF)  # [B, D, N]
    # fold ln_gamma into w_qkv rows; fold attention scale into the q columns
    w_eff = w_qkv * ln_gamma[:, None]
    qkvb_full = ln_beta @ w_qkv  # [3*H*DH]
    in_maps = []
    for c in range(NCORES):
        csl = slice(c * CL, (c + 1) * CL)
        wq = w_eff[:, 0:H * DH][:, csl] * SCALE
        wk = w_eff[:, H * DH:2 * H * DH][:, csl]
        wv = w_eff[:, 2 * H * DH:3 * H * DH][:, csl]
        wqkv_c = np.ascontiguousarray(np.concatenate([wq, wk, wv], axis=1)).astype(BF)
        nwsum_c = -wqkv_c.astype(np.float64).sum(axis=0).astype(np.float32)
        qb = qkvb_full.reshape(3, H * DH)[:, csl].copy()
        qb[0] *= SCALE
        qkvb_c = np.ascontiguousarray(qb.reshape(-1))
        al_c = np.ascontiguousarray(
            alibi_bias[0, c * HL:(c + 1) * HL].transpose(0, 2, 1)
        ).astype(BF)
        wout_c = np.ascontiguousarray(w_out[csl, :]).astype(BF)
        in_maps.append({
            "xt": xt,
            "alibi": al_c,
            "wqkv": wqkv_c,
            "qkvb": qkvb_c,
            "nwsum": nwsum_c,
            "wout": wout_c,
            "ones": np.ones((128, 1), BF),
        })
    return in_maps


def kernel(x, alibi_bias, mask, ln_gamma, ln_beta, w_qkv, w_out, _trace=False):
    global _CACHED_NC
    mask = np.asarray(mask)
    assert mask.all(), "kernel assumes an all-True mask"
    if _CACHED_NC is None:
        _CACHED_NC = build_nc()
    nc = _CACHED_NC
    in_maps = make_in_maps(x, alibi_bias, ln_gamma, ln_beta, w_qkv, w_out)
    res = run_bass_kernel_spmd(nc, in_maps, core_ids=list(range(NCORES)), trace=_trace)
    out_t = np.zeros((B, D, N), np.float32)
    for c in range(NCORES):
        out_t += res.results[c]["out"].astype(np.float32)
    out = np.ascontiguousarray(out_t.transpose(0, 2, 1))
    if _trace:
        return out, res
    return out


# revision 58
# speedup vs baseline: 1.3970x; 1.0156x over previous
"""Trainium2 Bass kernel for nn_Attention (LN -> QKV -> alibi attention -> out-proj).

Full shapes: x[2,2048,1024], alibi[1,16,2048,2048], w_qkv[1024,3072], w_out[1024,1024].
Sharding: tensor-parallel over heads. Core c owns heads {2c, 2c+1} for BOTH batches.
Each core computes a partial out-projection; the host sums the 8 partials (the
tensor-parallel reduction) and transposes back.

v2 design (all matmuls bf16 -- fp32r streams at 2 cyc/row on silicon, bf16 at 1):
  - x passed host-transposed + bf16: xT[b] = [d=1024, i=2048].
  - LN folded into the QKV eviction: qkv = rstd*(W^T x) + (mean*rstd)*(-colsum(W))
    (+ beta@W). LN stats (sum, sum-sq) via matmul-with-ones run concurrently with
    the QKV matmuls on raw x; no xn materialization, no LN->QKV serialization.
  - q/k evicted bf16 (2 heads on partitions); v bf16, PE-transposed to v-natural
    [j, 64d + ones-col]; the ones column makes attn@v also emit softmax denoms.
  - attention loops h outer, batch inner: each alibi^T tile (bf16, host-transposed)
    is DMA'd once and used by both batches (16MB/core alibi traffic, the minimum).
  - scores S^T = kT_chunk^T @ qT (K=64) + identity-matmul alibi accumulate; exp on
    ScalarE (PSUM f32 -> SBUF bf16), no max-subtraction (|scores| <~ 15).
  - PSUM: one shared [128,1024] pool (bufs=2) for stats/qkv/transpose/scores/
    out-proj + one [65,1024] pool (bufs=2) holding both batches' attn accumulators.
  - out partials written bf16 transposed [b, e, i]; host sums in f32.
"""

import sys

sys.path.insert(0, "/opt/trn_rl_repo")

from contextlib import ExitStack

import numpy as np
import ml_dtypes

import concourse.bass as bass
from concourse import bacc
import concourse.mybir as mybir
import concourse.tile as tile
from concourse.bass_utils import run_bass_kernel_spmd
from concourse.masks import make_identity

F32 = mybir.dt.float32
BF16 = mybir.dt.bfloat16

B, N, D = 2, 2048, 1024
H, DH = 16, 64
NCORES = 8
HL = H // NCORES          # local heads per core = 2
CL = HL * DH              # local head channels = 128
LN_EPS = 1e-5
SCALE = DH ** -0.5
KT = D // 128             # 8 d-tiles
JC = N // 128             # 16 j-chunks
IT = N // 512             # 4 i-tiles of 512

_CACHED_NC = None


def build_nc() -> bass.Bass:
    nc = bacc.Bacc(None)
    xt_d = nc.declare_dram_parameter("xt", [B, D, N], BF16, isOutput=False)
    al_d = nc.declare_dram_parameter("alibi", [HL, N, N], BF16, isOutput=False)
    wqkv_d = nc.declare_dram_parameter("wqkv", [D, 3 * CL], BF16, isOutput=False)
    qkvb_d = nc.declare_dram_parameter("qkvb", [3 * CL], F32, isOutput=False)
    nwsum_d = nc.declare_dram_parameter("nwsum", [3 * CL], F32, isOutput=False)
    wout_d = nc.declare_dram_parameter("wout", [CL, D], BF16, isOutput=False)
    ones_d = nc.declare_dram_parameter("ones", [128, 1], BF16, isOutput=False)
    out_d = nc.declare_dram_parameter("out", [B, D, N], BF16, isOutput=True)

    with tile.TileContext(nc) as tc, ExitStack() as ctx:
        ep = lambda **kw: ctx.enter_context(tc.tile_pool(**kw))
        cpool = ep(name="const", bufs=1)
        xt_pool = ep(name="xt", bufs=11)
        sq_pool = ep(name="sq", bufs=4)
        sm_pool = ep(name="small", bufs=2)
        tmp_pool = ep(name="tmp", bufs=4)
        qk_pool = ep(name="qk", bufs=1)      # per-batch tiles, all resident
        vt_pool = ep(name="vt", bufs=2)
        vn_pool = ep(name="vn", bufs=1)      # 4 resident tiles (b x head)
        al_pool = ep(name="al", bufs=8)
        at_pool = ep(name="at", bufs=8)
        ao_pool = ep(name="aos", bufs=1)
        ob_pool = ep(name="ob", bufs=4)
        bc_pool = ep(name="bc", bufs=4)
        rrbc_pool = ep(name="rrbc", bufs=3)
        aor_pool = ep(name="aor", bufs=3)
        dscr_pool = ep(name="dscr", bufs=2, space="DRAM")
        big_psum = ep(name="ps_big", bufs=3, space="PSUM")
        ao_psum = ep(name="ps_ao", bufs=1, space="PSUM")

        # ---- constants ----
        ident = cpool.tile([128, 128], BF16, name="ident")
        make_identity(nc, ident)
        zero_sb = cpool.tile([128, 1], F32, name="zero_sb")
        nc.vector.memset(zero_sb, 0.0)
        nc.const_aps.aps[(F32, 0.0)] = zero_sb[:, 0:1]
        eps_sb = cpool.tile([128, 1], F32, name="eps_sb")
        nc.vector.memset(eps_sb, LN_EPS)
        ones_sb = cpool.tile([128, 1], BF16, name="ones_sb")
        nc.sync.dma_start(out=ones_sb, in_=ones_d[:, :])
        wqkv_sb = cpool.tile([128, KT, 3 * CL], BF16, name="wqkv_sb")
        nc.sync.dma_start(out=wqkv_sb, in_=wqkv_d.rearrange("(t p) c -> p t c", p=128))
        qkvb_sb = cpool.tile([128, 3], F32, name="qkvb_sb")
        nc.sync.dma_start(out=qkvb_sb, in_=qkvb_d.rearrange("(c p) -> p c", p=128))
        nwsum_sb = cpool.tile([128, 3], F32, name="nwsum_sb")
        nc.sync.dma_start(out=nwsum_sb, in_=nwsum_d.rearrange("(c p) -> p c", p=128))
        wout_sb = cpool.tile([128, D], BF16, name="wout_sb")
        nc.sync.dma_start(out=wout_sb, in_=wout_d[:, :])

        qTs, kTs, vns, aos = [], [], [], []
        for b in range(B):
            # ---- load xT (bf16) ----
            xts = []
            for kt in range(KT):
                xt_t = xt_pool.tile([128, N], BF16, name=f"xt_{b}_{kt}", tag="xt")
                nc.sync.dma_start(out=xt_t, in_=xt_d[b, kt * 128:(kt + 1) * 128, :])
                xts.append(xt_t)

            # ---- LN stats (sum, sumsq) via matmul-with-ones ----

            scr = dscr_pool.tile([2, N], F32, name=f"scr_{b}", tag="scr")
            st = sm_pool.tile([128, 112], F32, name=f"st_{b}", tag="st128")
            for ihalf in range(2):
                isl = slice(ihalf * 1024, (ihalf + 1) * 1024)
                rows = sm_pool.tile([1, N], F32, name=f"rows_{b}_{ihalf}", tag="rows", bufs=1)
                sum_ps = big_psum.tile([1, 1024], F32, name=f"sum_{b}_{ihalf}", tag="big")
                sq_ps = big_psum.tile([33, 1024], F32, name=f"ssq_{b}_{ihalf}", tag="big")
                for kt in range(KT):
                    xsq = sq_pool.tile([128, 1024], BF16, name=f"xsq_{b}_{ihalf}_{kt}", tag="sq")
                    nc.vector.tensor_mul(xsq, xts[kt][:, isl], xts[kt][:, isl])
                    for it2 in range(2):
                        s2 = slice(it2 * 512, (it2 + 1) * 512)
                        i2 = slice(ihalf * 1024 + it2 * 512, ihalf * 1024 + (it2 + 1) * 512)
                        nc.tensor.matmul(
                            sum_ps[0:1, s2], ones_sb, xts[kt][:, i2],
                            start=(kt == 0), stop=(kt == KT - 1),
                        )
                        nc.tensor.matmul(
                            sq_ps[32:33, s2], ones_sb, xsq[:, s2],
                            start=(kt == 0), stop=(kt == KT - 1),
                            tile_position=(0, 32),
                        )
                nc.vector.tensor_copy(rows[0:1, 0:1024], sum_ps)
                nc.vector.tensor_copy(rows[0:1, 1024:2048], sq_ps[32:33, :])
                nc.sync.dma_start(out=st[:, ihalf * 8:(ihalf + 1) * 8], in_=rows[0:1, 0:1024])
                nc.sync.dma_start(out=st[:, 16 + ihalf * 8:16 + (ihalf + 1) * 8], in_=rows[0:1, 1024:2048])
            # stat cols: 0:16 sum128, 16:32 sumsq128, 32:48 mean, 48:64 ex2,
            # 64:80 -var, 80:96 rstd, 96:112 mean*rstd
            mean, ex2 = st[:, 32:48], st[:, 48:64]
            nvar, rstd, mrs = st[:, 64:80], st[:, 80:96], st[:, 96:112]
            nc.vector.tensor_scalar_mul(mean, st[:, 0:16], 1.0 / D)
            nc.vector.tensor_scalar_mul(ex2, st[:, 16:32], 1.0 / D)
            nc.vector.tensor_mul(nvar, mean, mean)
            nc.vector.tensor_sub(nvar, nvar, ex2)  # mean^2 - E[x^2] = -var
            nc.scalar.activation(
                rstd, nvar, mybir.ActivationFunctionType.Sqrt,
                bias=eps_sb[:, 0:1], scale=-1.0,
            )  # sqrt(var + eps)
            nc.vector.reciprocal(rstd, rstd)
            nc.vector.tensor_mul(mrs, mean, rstd)
            for ihalf in range(2):
                c8 = slice(ihalf * 8, (ihalf + 1) * 8)
                isl = slice(ihalf * 1024, (ihalf + 1) * 1024)
                nc.sync.dma_start(out=scr[0:1, isl], in_=rstd[:, c8])
                nc.sync.dma_start(out=scr[1:2, isl], in_=mrs[:, c8])
            # broadcast rows (DRAM -> 128 partitions)
            bcs = []
            for ihalf in range(2):
                isl = slice(ihalf * 1024, (ihalf + 1) * 1024)
                rstd_bc = bc_pool.tile([128, 1024], F32, name=f"rsbc_{b}_{ihalf}", tag="bc")
                nc.sync.dma_start(out=rstd_bc, in_=scr[0:1, isl].partition_broadcast(128))
                mrs_bc = bc_pool.tile([128, 1024], F32, name=f"mrbc_{b}_{ihalf}", tag="bc")
                nc.sync.dma_start(out=mrs_bc, in_=scr[1:2, isl].partition_broadcast(128))
                bcs.append((rstd_bc, mrs_bc))

            # ---- QKV projection on raw x; LN affine applied at eviction ----
            qT = qk_pool.tile([128, N], BF16, name=f"qT_{b}", tag=f"qT{b}")
            kT = qk_pool.tile([128, N], BF16, name=f"kT_{b}", tag=f"kT{b}")
            vT = vt_pool.tile([128, N], BF16, name=f"vT_{b}", tag="vT")
            qTs.append(qT)
            kTs.append(kT)
            sb_dst = [qT, kT, vT]
            for cc in (1, 2, 0):
                for ihalf in range(2):
                    isl = slice(ihalf * 1024, (ihalf + 1) * 1024)
                    pt = big_psum.tile([128, 1024], F32, name=f"qp_{b}_{cc}_{ihalf}", tag="big")
                    for kt in range(KT):
                        lhs = wqkv_sb[:, kt, cc * 128:(cc + 1) * 128]
                        for it2 in range(2):
                            s2 = slice(it2 * 512, (it2 + 1) * 512)
                            i2 = slice(ihalf * 1024 + it2 * 512, ihalf * 1024 + (it2 + 1) * 512)
                            bi = nc.tensor.matmul(
                                pt[:, s2], lhs, xts[kt][:, i2],
                                start=(kt == 0), stop=(kt == KT - 1),
                            )
                            if it2 == 1:
                                bi.ins.ldweights = False
                    rstd_bc, mrs_bc = bcs[ihalf]
                    tmp = tmp_pool.tile([128, 1024], F32, name=f"tmp_{b}_{cc}_{ihalf}", tag="tmp")
                    nc.vector.tensor_mul(tmp, pt, rstd_bc)
                    nc.vector.scalar_tensor_tensor(
                        out=sb_dst[cc][:, isl], in0=mrs_bc,
                        scalar=nwsum_sb[:, cc:cc + 1], in1=tmp,
                        op0=mybir.AluOpType.mult, op1=mybir.AluOpType.add,
                    )
                    nc.vector.tensor_scalar_add(
                        sb_dst[cc][:, isl], sb_dst[cc][:, isl], qkvb_sb[:, cc:cc + 1]
                    )

            # ---- v natural (+ ones column) via PE transpose ----
            vb = []
            for hh in range(HL):
                vn = vn_pool.tile([128, JC, DH + 1], BF16, name=f"vn_{b}_{hh}", tag=f"vn{b}{hh}")
                nc.gpsimd.memset(vn[:, :, DH:DH + 1], 1.0)
                vb.append(vn)
            vns.append(vb)
            for jc in range(JC):
                trp = big_psum.tile([128, 128], BF16, name=f"tr_{b}_{jc}", tag="big")
                nc.tensor.transpose(trp, vT[:, jc * 128:(jc + 1) * 128], ident)
                for hh in range(HL):
                    nc.vector.tensor_copy(
                        vb[hh][:, jc, 0:DH], trp[:, hh * DH:(hh + 1) * DH]
                    )

            ao_sb = ao_pool.tile([128, N], BF16, name=f"ao_{b}", tag=f"ao{b}")
            aos.append(ao_sb)

        # ---- attention: batch outer; ONE psum accumulator per group so two
        #      score tiles are in flight (PE never waits on ScalarE exp) ----
        scr3 = dscr_pool.tile([16, 1024], F32, name="scr3", tag="scr3")
        scr4 = dscr_pool.tile([16, 1024], F32, name="scr4", tag="scr4")
        for b in range(B):
            for hh in range(HL):
                hsl = slice(hh * DH, (hh + 1) * DH)
                for ihalf in range(2):
                    isl = slice(ihalf * 1024, (ihalf + 1) * 1024)
                    aop = ao_psum.tile([DH + 1, 1024], F32, name=f"aop_{b}_{hh}_{ihalf}", tag="aop")
                    for jc in range(JC):
                        jsl = slice(jc * 128, (jc + 1) * 128)
                        al_t = al_pool.tile([128, 1024], BF16, name=f"al_{b}_{hh}_{ihalf}_{jc}", tag="al")
                        nc.sync.dma_start(out=al_t, in_=al_d[hh, jsl, isl])
                        sc = big_psum.tile([128, 1024], F32, name=f"sc_{b}_{hh}_{ihalf}_{jc}", tag="big")
                        for it2 in range(2):
                            s2 = slice(it2 * 512, (it2 + 1) * 512)
                            i2 = slice(ihalf * 1024 + it2 * 512, ihalf * 1024 + (it2 + 1) * 512)
                            bi = nc.tensor.matmul(
                                sc[:, s2], kTs[b][hsl, jsl], qTs[b][hsl, i2],
                                start=True, stop=(it2 == 1),
                            )
                            if it2 == 1:
                                bi.ins.ldweights = False
                        nc.tensor.matmul(
                            sc[:, 0:512], ident, al_t[:, 0:512],
                            start=False, stop=True,
                        )
                        nc.vector.tensor_add(sc[:, 512:1024], sc[:, 512:1024], al_t[:, 512:1024])
                        at_t = at_pool.tile([128, 1024], BF16, name=f"at_{b}_{hh}_{ihalf}_{jc}", tag="at")
                        nc.scalar.activation(at_t, sc, mybir.ActivationFunctionType.Exp)
                        for it2 in range(2):
                            s2 = slice(it2 * 512, (it2 + 1) * 512)
                            bi = nc.tensor.matmul(
                                aop[:, s2], vns[b][hh][:, jc, :], at_t[:, s2],
                                start=(jc == 0), stop=(jc == JC - 1),
                            )
                            if it2 == 1:
                                bi.ins.ldweights = False
                    # evict raw attn output immediately (frees PSUM); the
                    # reciprocal runs at [128,8] via DMA reshapes and the
                    # normalize happens off the critical path
                    r = (hh * 2 + ihalf) * 2 + b
                    ao_raw = aor_pool.tile([DH + 1, 1024], F32, name=f"aor_{r}", tag="aor")
                    nc.vector.tensor_copy(ao_raw, aop)
                    nc.sync.dma_start(out=scr3[r:r + 1, :], in_=ao_raw[DH:DH + 1, :])
                    r128 = sm_pool.tile([128, 8], F32, name=f"r128_{r}", tag="r128", bufs=3)
                    nc.sync.dma_start(out=r128, in_=scr3[r:r + 1, :])
                    nc.vector.reciprocal(r128, r128)
                    nc.sync.dma_start(out=scr4[r:r + 1, :], in_=r128)
                    rr_bc = rrbc_pool.tile([DH, 1024], F32, name=f"rrbc_{r}", tag="rrbc")
                    nc.sync.dma_start(
                        out=rr_bc, in_=scr4[r:r + 1, :].partition_broadcast(DH)
                    )
                    nc.vector.tensor_mul(aos[b][hsl, isl], ao_raw[0:DH, :], rr_bc)

        # ---- out projection (partial, transposed, bf16) ----
        for ihalf in range(2):
            for b in range(B):
                for ec in range(8):
                    lhs = wout_sb[:, ec * 128:(ec + 1) * 128]
                    isl = slice(ihalf * 1024, (ihalf + 1) * 1024)
                    opp = big_psum.tile([128, 1024], F32, name=f"op_{b}_{ec}_{ihalf}", tag="big")
                    for it2 in range(2):
                        s2 = slice(it2 * 512, (it2 + 1) * 512)
                        i2 = slice(ihalf * 1024 + it2 * 512, ihalf * 1024 + (it2 + 1) * 512)
                        bi = nc.tensor.matmul(opp[:, s2], lhs, aos[b][:, i2], start=True, stop=True)
                        if it2 == 1:
                            bi.ins.ldweights = False
                    ob = ob_pool.tile([128, 1024], BF16, name=f"ob_{b}_{ec}_{ihalf}", tag="ob")
                    nc.vector.tensor_copy(ob, opp)
                    nc.sync.dma_start(out=out_d[b, ec * 128:(ec + 1) * 128, isl], in_=ob)
    nc.compile()
    return nc


def make_in_maps(x, alibi_bias, ln_gamma, ln_beta, w_qkv, w_out):
    """Host-side sharding / layout prep. Returns list of 8 per-core input dicts."""
    x = np.asarray(x, np.float32)
    alibi_bias = np.asarray(alibi_bias, np.float32)
    ln_gamma = np.asarray(ln_gamma, np.float32)
    ln_beta = np.asarray(ln_beta, np.float32)
    w_qkv = np.asarray(w_qkv, np.float32)
    w_out = np.asarray(w_out, np.float32)
    BF = ml_dtypes.bfloat16

    xt = np.ascontiguousarray(x.transpose(0, 2, 1)).astype(BF)  # [B, D, N]
    # fold ln_gamma into w_qkv rows; fold attention scale into the q columns
    w_eff = w_qkv * ln_gamma[:, None]
    qkvb_full = ln_beta @ w_qkv  # [3*H*DH]
    in_maps = []
    for c in range(NCORES):
        csl = slice(c * CL, (c + 1) * CL)
        wq = w_eff[:, 0:H * DH][:, csl] * SCALE
        wk = w_eff[:, H * DH:2 * H * DH][:, csl]
        wv = w_eff[:, 2 * H * DH:3 * H * DH][:, csl]
        wqkv_c = np.ascontiguousarray(np.concatenate([wq, wk, wv], axis=1)).astype(BF)
        nwsum_c = -wqkv_c.astype(np.float64).sum(axis=0).astype(np.float32)
        qb = qkvb_full.reshape(3, H * DH)[:, csl].copy()
        qb[0] *= SCALE
        qkvb_c = np.ascontiguousarray(qb.reshape(-1))
        al_c = np.ascontiguousarray(
            alibi_bias[0, c * HL:(c + 1) * HL].transpose(0, 2, 1)
        ).astype(BF)
        wout_c = np.ascontiguousarray(w_out[csl, :]).astype(BF)
        in_maps.append({
            "xt": xt,
            "alibi": al_c,
            "wqkv": wqkv_c,
            "qkvb": qkvb_c,
            "nwsum": nwsum_c,
            "wout": wout_c,
            "ones": np.ones((128, 1), BF),
        })
    return in_maps


def kernel(x, alibi_bias, mask, ln_gamma, ln_beta, w_qkv, w_out, _trace=False):
    global _CACHED_NC
    mask = np.asarray(mask)
    assert mask.all(), "kernel assumes an all-True mask"
    if _CACHED_NC is None:
        _CACHED_NC = build_nc()
    nc = _CACHED_NC
    in_maps = make_in_maps(x, alibi_bias, ln_gamma, ln_beta, w_qkv, w_out)
    res = run_bass_kernel_spmd(nc, in_maps, core_ids=list(range(NCORES)), trace=_trace)
    out_t = np.zeros((B, D, N), np.float32)
    for c in range(NCORES):
        out_t += res.results[c]["out"].astype(np.float32)
    out = np.ascontiguousarray(out_t.transpose(0, 2, 1))
    if _trace:
        return out, res
    return out


# revision 61
# speedup vs baseline: 1.3986x; 1.0012x over previous
"""Trainium2 Bass kernel for nn_Attention (LN -> QKV -> alibi attention -> out-proj).

Full shapes: x[2,2048,1024], alibi[1,16,2048,2048], w_qkv[1024,3072], w_out[1024,1024].
Sharding: tensor-parallel over heads. Core c owns heads {2c, 2c+1} for BOTH batches.
Each core computes a partial out-projection; the host sums the 8 partials (the
tensor-parallel reduction) and transposes back.

v2 design (all matmuls bf16 -- fp32r streams at 2 cyc/row on silicon, bf16 at 1):
  - x passed host-transposed + bf16: xT[b] = [d=1024, i=2048].
  - LN folded into the QKV eviction: qkv = rstd*(W^T x) + (mean*rstd)*(-colsum(W))
    (+ beta@W). LN stats (sum, sum-sq) via matmul-with-ones run concurrently with
    the QKV matmuls on raw x; no xn materialization, no LN->QKV serialization.
  - q/k evicted bf16 (2 heads on partitions); v bf16, PE-transposed to v-natural
    [j, 64d + ones-col]; the ones column makes attn@v also emit softmax denoms.
  - attention loops h outer, batch inner: each alibi^T tile (bf16, host-transposed)
    is DMA'd once and used by both batches (16MB/core alibi traffic, the minimum).
  - scores S^T = kT_chunk^T @ qT (K=64) + identity-matmul alibi accumulate; exp on
    ScalarE (PSUM f32 -> SBUF bf16), no max-subtraction (|scores| <~ 15).
  - PSUM: one shared [128,1024] pool (bufs=2) for stats/qkv/transpose/scores/
    out-proj + one [65,1024] pool (bufs=2) holding both batches' attn accumulators.
  - out partials written bf16 transposed [b, e, i]; host sums in f32.
"""

import sys

sys.path.insert(0, "/opt/trn_rl_repo")

from contextlib import ExitStack

import numpy as np
import ml_dtypes

import concourse.bass as bass
from concourse import bacc
import concourse.mybir as mybir
import concourse.tile as tile
from concourse.bass_utils import run_bass_kernel_spmd
from concourse.masks import make_identity

F32 = mybir.dt.float32
BF16 = mybir.dt.bfloat16

B, N, D = 2, 2048, 1024
H, DH = 16, 64
NCORES = 8
HL = H // NCORES          # local heads per core = 2
CL = HL * DH              # local head channels = 128
LN_EPS = 1e-5
SCALE = DH ** -0.5
KT = D // 128             # 8 d-tiles
JC = N // 128             # 16 j-chunks
IT = N // 512             # 4 i-tiles of 512

_CACHED_NC = None


def build_nc() -> bass.Bass:
    nc = bacc.Bacc(None)
    xt_d = nc.declare_dram_parameter("xt", [B, D, N], BF16, isOutput=False)
    al_d = nc.declare_dram_parameter("alibi", [HL, N, N], BF16, isOutput=False)
    wqkv_d = nc.declare_dram_parameter("wqkv", [D, 3 * CL], BF16, isOutput=False)
    qkvb_d = nc.declare_dram_parameter("qkvb", [3 * CL], F32, isOutput=False)
    nwsum_d = nc.declare_dram_parameter("nwsum", [3 * CL], F32, isOutput=False)
    wout_d = nc.declare_dram_parameter("wout", [CL, D], BF16, isOutput=False)
    ones_d = nc.declare_dram_parameter("ones", [128, 1], BF16, isOutput=False)
    out_d = nc.declare_dram_parameter("out", [B, D, N], BF16, isOutput=True)

    with tile.TileContext(nc) as tc, ExitStack() as ctx:
        ep = lambda **kw: ctx.enter_context(tc.tile_pool(**kw))
        cpool = ep(name="const", bufs=1)
        xt_pool = ep(name="xt", bufs=11)
        sq_pool = ep(name="sq", bufs=4)
        sm_pool = ep(name="small", bufs=2)
        tmp_pool = ep(name="tmp", bufs=4)
        qk_pool = ep(name="qk", bufs=1)      # per-batch tiles, all resident
        vt_pool = ep(name="vt", bufs=2)
        vn_pool = ep(name="vn", bufs=1)      # 4 resident tiles (b x head)
        al_pool = ep(name="al", bufs=8)
        at_pool = ep(name="at", bufs=8)
        ao_pool = ep(name="aos", bufs=1)
        ob_pool = ep(name="ob", bufs=4)
        bc_pool = ep(name="bc", bufs=4)
        rrbc_pool = ep(name="rrbc", bufs=3)
        aor_pool = ep(name="aor", bufs=3)
        dscr_pool = ep(name="dscr", bufs=2, space="DRAM")
        big_psum = ep(name="ps_big", bufs=3, space="PSUM")
        ao_psum = ep(name="ps_ao", bufs=1, space="PSUM")

        # ---- constants ----
        ident = cpool.tile([128, 128], BF16, name="ident")
        make_identity(nc, ident)
        zero_sb = cpool.tile([128, 1], F32, name="zero_sb")
        nc.vector.memset(zero_sb, 0.0)
        nc.const_aps.aps[(F32, 0.0)] = zero_sb[:, 0:1]
        eps_sb = cpool.tile([128, 1], F32, name="eps_sb")
        nc.vector.memset(eps_sb, LN_EPS)
        ones_sb = cpool.tile([128, 1], BF16, name="ones_sb")
        nc.sync.dma_start(out=ones_sb, in_=ones_d[:, :])
        wqkv_sb = cpool.tile([128, KT, 3 * CL], BF16, name="wqkv_sb")
        nc.sync.dma_start(out=wqkv_sb, in_=wqkv_d.rearrange("(t p) c -> p t c", p=128))
        qkvb_sb = cpool.tile([128, 3], F32, name="qkvb_sb")
        nc.sync.dma_start(out=qkvb_sb, in_=qkvb_d.rearrange("(c p) -> p c", p=128))
        nwsum_sb = cpool.tile([128, 3], F32, name="nwsum_sb")
        nc.sync.dma_start(out=nwsum_sb, in_=nwsum_d.rearrange("(c p) -> p c", p=128))
        wout_sb = cpool.tile([128, D], BF16, name="wout_sb")
        nc.sync.dma_start(out=wout_sb, in_=wout_d[:, :])

        qTs, kTs, vns, aos = [], [], [], []
        for b in range(B):
            # ---- load xT (bf16) ----
            xts = []
            for kt in range(KT):
                xt_t = xt_pool.tile([128, N], BF16, name=f"xt_{b}_{kt}", tag="xt")
                nc.sync.dma_start(out=xt_t, in_=xt_d[b, kt * 128:(kt + 1) * 128, :])
                xts.append(xt_t)

            # ---- LN stats (sum, sumsq) via matmul-with-ones ----

            scr = dscr_pool.tile([2, N], F32, name=f"scr_{b}", tag="scr")
            st = sm_pool.tile([128, 112], F32, name=f"st_{b}", tag="st128")
            for ihalf in range(2):
                isl = slice(ihalf * 1024, (ihalf + 1) * 1024)
                rows = sm_pool.tile([1, N], F32, name=f"rows_{b}_{ihalf}", tag="rows", bufs=1)
                sum_ps = big_psum.tile([1, 1024], F32, name=f"sum_{b}_{ihalf}", tag="big")
                sq_ps = big_psum.tile([33, 1024], F32, name=f"ssq_{b}_{ihalf}", tag="big")
                for kt in range(KT):
                    xsq = sq_pool.tile([128, 1024], BF16, name=f"xsq_{b}_{ihalf}_{kt}", tag="sq")
                    nc.vector.tensor_mul(xsq, xts[kt][:, isl], xts[kt][:, isl])
                    for it2 in range(2):
                        s2 = slice(it2 * 512, (it2 + 1) * 512)
                        i2 = slice(ihalf * 1024 + it2 * 512, ihalf * 1024 + (it2 + 1) * 512)
                        nc.tensor.matmul(
                            sum_ps[0:1, s2], ones_sb, xts[kt][:, i2],
                            start=(kt == 0), stop=(kt == KT - 1),
                        )
                        nc.tensor.matmul(
                            sq_ps[32:33, s2], ones_sb, xsq[:, s2],
                            start=(kt == 0), stop=(kt == KT - 1),
                            tile_position=(0, 32),
                        )
                nc.vector.tensor_copy(rows[0:1, 0:1024], sum_ps)
                nc.vector.tensor_copy(rows[0:1, 1024:2048], sq_ps[32:33, :])
                nc.sync.dma_start(out=st[:, ihalf * 8:(ihalf + 1) * 8], in_=rows[0:1, 0:1024])
                nc.sync.dma_start(out=st[:, 16 + ihalf * 8:16 + (ihalf + 1) * 8], in_=rows[0:1, 1024:2048])
            # stat cols: 0:16 sum128, 16:32 sumsq128, 32:48 mean, 48:64 ex2,
            # 64:80 -var, 80:96 rstd, 96:112 mean*rstd
            mean, ex2 = st[:, 32:48], st[:, 48:64]
            nvar, rstd, mrs = st[:, 64:80], st[:, 80:96], st[:, 96:112]
            nc.vector.tensor_scalar_mul(mean, st[:, 0:16], 1.0 / D)
            nc.vector.tensor_scalar_mul(ex2, st[:, 16:32], 1.0 / D)
            nc.vector.tensor_mul(nvar, mean, mean)
            nc.vector.tensor_sub(nvar, nvar, ex2)  # mean^2 - E[x^2] = -var
            nc.scalar.activation(
                rstd, nvar, mybir.ActivationFunctionType.Sqrt,
                bias=eps_sb[:, 0:1], scale=-1.0,
            )  # sqrt(var + eps)
            nc.vector.reciprocal(rstd, rstd)
            nc.vector.tensor_mul(mrs, mean, rstd)
            for ihalf in range(2):
                c8 = slice(ihalf * 8, (ihalf + 1) * 8)
                isl = slice(ihalf * 1024, (ihalf + 1) * 1024)
                nc.sync.dma_start(out=scr[0:1, isl], in_=rstd[:, c8])
                nc.sync.dma_start(out=scr[1:2, isl], in_=mrs[:, c8])
            # broadcast rows (DRAM -> 128 partitions)
            bcs = []
            for ihalf in range(2):
                isl = slice(ihalf * 1024, (ihalf + 1) * 1024)
                rstd_bc = bc_pool.tile([128, 1024], F32, name=f"rsbc_{b}_{ihalf}", tag="bc")
                nc.sync.dma_start(out=rstd_bc, in_=scr[0:1, isl].partition_broadcast(128))
                mrs_bc = bc_pool.tile([128, 1024], F32, name=f"mrbc_{b}_{ihalf}", tag="bc")
                nc.sync.dma_start(out=mrs_bc, in_=scr[1:2, isl].partition_broadcast(128))
                bcs.append((rstd_bc, mrs_bc))

            # ---- QKV projection on raw x; LN affine applied at eviction ----
            qT = qk_pool.tile([128, N], BF16, name=f"qT_{b}", tag=f"qT{b}")
            kT = qk_pool.tile([128, N], BF16, name=f"kT_{b}", tag=f"kT{b}")
            vT = vt_pool.tile([128, N], BF16, name=f"vT_{b}", tag="vT")
            qTs.append(qT)
            kTs.append(kT)
            sb_dst = [qT, kT, vT]
            for cc in (1, 2, 0):
                for ihalf in range(2):
                    isl = slice(ihalf * 1024, (ihalf + 1) * 1024)
                    pt = big_psum.tile([128, 1024], F32, name=f"qp_{b}_{cc}_{ihalf}", tag="big")
                    for kt in range(KT):
                        lhs = wqkv_sb[:, kt, cc * 128:(cc + 1) * 128]
                        for it2 in range(2):
                            s2 = slice(it2 * 512, (it2 + 1) * 512)
                            i2 = slice(ihalf * 1024 + it2 * 512, ihalf * 1024 + (it2 + 1) * 512)
                            bi = nc.tensor.matmul(
                                pt[:, s2], lhs, xts[kt][:, i2],
                                start=(kt == 0), stop=(kt == KT - 1),
                            )
                            if it2 == 1:
                                bi.ins.ldweights = False
                    rstd_bc, mrs_bc = bcs[ihalf]
                    tmp = tmp_pool.tile([128, 1024], F32, name=f"tmp_{b}_{cc}_{ihalf}", tag="tmp")
                    nc.vector.tensor_mul(tmp, pt, rstd_bc)
                    nc.vector.scalar_tensor_tensor(
                        out=sb_dst[cc][:, isl], in0=mrs_bc,
                        scalar=nwsum_sb[:, cc:cc + 1], in1=tmp,
                        op0=mybir.AluOpType.mult, op1=mybir.AluOpType.add,
                    )
                    nc.vector.tensor_scalar_add(
                        sb_dst[cc][:, isl], sb_dst[cc][:, isl], qkvb_sb[:, cc:cc + 1]
                    )

            # ---- v natural (+ ones column) via PE transpose ----
            vb = []
            for hh in range(HL):
                vn = vn_pool.tile([128, JC, DH + 1], BF16, name=f"vn_{b}_{hh}", tag=f"vn{b}{hh}")
                nc.gpsimd.memset(vn[:, :, DH:DH + 1], 1.0)
                vb.append(vn)
            vns.append(vb)
            for jc in range(JC):
                trp = big_psum.tile([128, 128], BF16, name=f"tr_{b}_{jc}", tag="big")
                nc.tensor.transpose(trp, vT[:, jc * 128:(jc + 1) * 128], ident)
                for hh in range(HL):
                    nc.vector.tensor_copy(
                        vb[hh][:, jc, 0:DH], trp[:, hh * DH:(hh + 1) * DH]
                    )

            ao_sb = ao_pool.tile([128, N], BF16, name=f"ao_{b}", tag=f"ao{b}")
            aos.append(ao_sb)

        # ---- attention: batch outer; ONE psum accumulator per group so two
        #      score tiles are in flight (PE never waits on ScalarE exp) ----
        scr3 = dscr_pool.tile([16, 1024], F32, name="scr3", tag="scr3")
        scr4 = dscr_pool.tile([16, 1024], F32, name="scr4", tag="scr4")
        for b in range(B):
            for hh in range(HL):
                hsl = slice(hh * DH, (hh + 1) * DH)
                for ihalf in range(2):
                    isl = slice(ihalf * 1024, (ihalf + 1) * 1024)
                    aop = ao_psum.tile([DH + 1, 1024], F32, name=f"aop_{b}_{hh}_{ihalf}", tag="aop")
                    for jc in range(JC):
                        jsl = slice(jc * 128, (jc + 1) * 128)
                        al_t = al_pool.tile([128, 1024], BF16, name=f"al_{b}_{hh}_{ihalf}_{jc}", tag="al")
                        nc.sync.dma_start(out=al_t, in_=al_d[hh, jsl, isl])
                        sc = big_psum.tile([128, 1024], F32, name=f"sc_{b}_{hh}_{ihalf}_{jc}", tag="big")
                        for it2 in range(2):
                            s2 = slice(it2 * 512, (it2 + 1) * 512)
                            i2 = slice(ihalf * 1024 + it2 * 512, ihalf * 1024 + (it2 + 1) * 512)
                            bi = nc.tensor.matmul(
                                sc[:, s2], kTs[b][hsl, jsl], qTs[b][hsl, i2],
                                start=True, stop=(it2 == 1),
                            )
                            if it2 == 1:
                                bi.ins.ldweights = False
                        nc.tensor.matmul(
                            sc[:, 0:512], ident, al_t[:, 0:512],
                            start=False, stop=True,
                        )
                        nc.vector.tensor_add(sc[:, 512:1024], sc[:, 512:1024], al_t[:, 512:1024])
                        at_t = at_pool.tile([128, 1024], BF16, name=f"at_{b}_{hh}_{ihalf}_{jc}", tag="at")
                        nc.scalar.activation(at_t, sc, mybir.ActivationFunctionType.Exp)
                        for it2 in range(2):
                            s2 = slice(it2 * 512, (it2 + 1) * 512)
                            bi = nc.tensor.matmul(
                                aop[:, s2], vns[b][hh][:, jc, :], at_t[:, s2],
                                start=(jc == 0), stop=(jc == JC - 1),
                            )
                            if it2 == 1:
                                bi.ins.ldweights = False
                    # evict raw attn output immediately (frees PSUM); the
                    # reciprocal runs at [128,8] via DMA reshapes and the
                    # normalize happens off the critical path
                    r = (hh * 2 + ihalf) * 2 + b
                    ao_raw = aor_pool.tile([DH + 1, 1024], F32, name=f"aor_{r}", tag="aor")
                    nc.vector.tensor_copy(ao_raw, aop)
                    nc.sync.dma_start(out=scr3[r:r + 1, :], in_=ao_raw[DH:DH + 1, :])
                    r128 = sm_pool.tile([128, 8], F32, name=f"r128_{r}", tag="r128", bufs=3)
                    nc.sync.dma_start(out=r128, in_=scr3[r:r + 1, :])
                    nc.vector.reciprocal(r128, r128)
                    nc.sync.dma_start(out=scr4[r:r + 1, :], in_=r128)
                    rr_bc = rrbc_pool.tile([DH, 1024], F32, name=f"rrbc_{r}", tag="rrbc")
                    nc.sync.dma_start(
                        out=rr_bc, in_=scr4[r:r + 1, :].partition_broadcast(DH)
                    )
                    nc.vector.tensor_mul(aos[b][hsl, isl], ao_raw[0:DH, :], rr_bc)

        # ---- out projection (partial, transposed, bf16) ----
        for ihalf in range(2):
            for b in range(B):
                for ec in range(8):
                    lhs = wout_sb[:, ec * 128:(ec + 1) * 128]
                    isl = slice(ihalf * 1024, (ihalf + 1) * 1024)
                    opp = big_psum.tile([128, 1024], F32, name=f"op_{b}_{ec}_{ihalf}", tag="big")
                    for it2 in range(2):
                        s2 = slice(it2 * 512, (it2 + 1) * 512)
                        i2 = slice(ihalf * 1024 + it2 * 512, ihalf * 1024 + (it2 + 1) * 512)
                        bi = nc.tensor.matmul(opp[:, s2], lhs, aos[b][:, i2], start=True, stop=True)
                        if it2 == 1:
                            bi.ins.ldweights = False
                    ob = ob_pool.tile([128, 1024], BF16, name=f"ob_{b}_{ec}_{ihalf}", tag="ob")
                    nc.vector.tensor_copy(ob, opp)
                    nc.sync.dma_start(out=out_d[b, ec * 128:(ec + 1) * 128, isl], in_=ob)
    nc.compile()
    return nc


def make_in_maps(x, alibi_bias, ln_gamma, ln_beta, w_qkv, w_out):
    """Host-side sharding / layout prep. Returns list of 8 per-core input dicts."""
    x = np.asarray(x, np.float32)
    alibi_bias = np.asarray(alibi_bias, np.float32)
    ln_gamma = np.asarray(ln_gamma, np.float32)
    ln_beta = np.asarray(ln_beta, np.float32)
    w_qkv = np.asarray(w_qkv, np.float32)
    w_out = np.asarray(w_out, np.float32)
    BF = ml_dtypes.bfloat16

    xt = np.ascontiguousarray(x.transpose(0, 2, 1)).astype(BF)  # [B, D, N]
    # fold ln_gamma into w_qkv rows; fold attention scale into the q columns
    w_eff = w_qkv * ln_gamma[:, None]
    qkvb_full = ln_beta @ w_qkv  # [3*H*DH]
    in_maps = []
    for c in range(NCORES):
        csl = slice(c * CL, (c + 1) * CL)
        wq = w_eff[:, 0:H * DH][:, csl] * SCALE
        wk = w_eff[:, H * DH:2 * H * DH][:, csl]
        wv = w_eff[:, 2 * H * DH:3 * H * DH][:, csl]
        wqkv_c = np.ascontiguousarray(np.concatenate([wq, wk, wv], axis=1)).astype(BF)
        nwsum_c = -wqkv_c.astype(np.float64).sum(axis=0).astype(np.float32)
        qb = qkvb_full.reshape(3, H * DH)[:, csl].copy()
        qb[0] *= SCALE
        qkvb_c = np.ascontiguousarray(qb.reshape(-1))
        al_c = np.ascontiguousarray(
            alibi_bias[0, c * HL:(c + 1) * HL].transpose(0, 2, 1)
        ).astype(BF)
        wout_c = np.ascontiguousarray(w_out[csl, :]).astype(BF)
        in_maps.append({
            "xt": xt,
            "alibi": al_c,
            "wqkv": wqkv_c,
            "qkvb": qkvb_c,
            "nwsum": nwsum_c,
            "wout": wout_c,
            "ones": np.ones((128, 1), BF),
        })
    return in_maps


def kernel(x, alibi_bias, mask, ln_gamma, ln_beta, w_qkv, w_out, _trace=False):
    global _CACHED_NC
    mask = np.asarray(mask)
    assert mask.all(), "kernel assumes an all-True mask"
    if _CACHED_NC is None:
        _CACHED_NC = build_nc()
    nc = _CACHED_NC
    in_maps = make_in_maps(x, alibi_bias, ln_gamma, ln_beta, w_qkv, w_out)
    res = run_bass_kernel_spmd(nc, in_maps, core_ids=list(range(NCORES)), trace=_trace)
    out_t = np.zeros((B, D, N), np.float32)
    for c in range(NCORES):
        out_t += res.results[c]["out"].astype(np.float32)
    out = np.ascontiguousarray(out_t.transpose(0, 2, 1))
    if _trace:
        return out, res
    return out


# revision 62
# speedup vs baseline: 1.4196x; 1.0150x over previous
"""Trainium2 Bass kernel for nn_Attention (LN -> QKV -> alibi attention -> out-proj).

Full shapes: x[2,2048,1024], alibi[1,16,2048,2048], w_qkv[1024,3072], w_out[1024,1024].
Sharding: tensor-parallel over heads. Core c owns heads {2c, 2c+1} for BOTH batches.
Each core computes a partial out-projection; the host sums the 8 partials (the
tensor-parallel reduction) and transposes back.

v2 design (all matmuls bf16 -- fp32r streams at 2 cyc/row on silicon, bf16 at 1):
  - x passed host-transposed + bf16: xT[b] = [d=1024, i=2048].
  - LN folded into the QKV eviction: qkv = rstd*(W^T x) + (mean*rstd)*(-colsum(W))
    (+ beta@W). LN stats (sum, sum-sq) via matmul-with-ones run concurrently with
    the QKV matmuls on raw x; no xn materialization, no LN->QKV serialization.
  - q/k evicted bf16 (2 heads on partitions); v bf16, PE-transposed to v-natural
    [j, 64d + ones-col]; the ones column makes attn@v also emit softmax denoms.
  - attention loops h outer, batch inner: each alibi^T tile (bf16, host-transposed)
    is DMA'd once and used by both batches (16MB/core alibi traffic, the minimum).
  - scores S^T = kT_chunk^T @ qT (K=64) + identity-matmul alibi accumulate; exp on
    ScalarE (PSUM f32 -> SBUF bf16), no max-subtraction (|scores| <~ 15).
  - PSUM: one shared [128,1024] pool (bufs=2) for stats/qkv/transpose/scores/
    out-proj + one [65,1024] pool (bufs=2) holding both batches' attn accumulators.
  - out partials written bf16 transposed [b, e, i]; host sums in f32.
"""

import sys

sys.path.insert(0, "/opt/trn_rl_repo")

from contextlib import ExitStack

import numpy as np
import ml_dtypes

import concourse.bass as bass
from concourse import bacc
import concourse.mybir as mybir
import concourse.tile as tile
from concourse.bass_utils import run_bass_kernel_spmd
from concourse.masks import make_identity

F32 = mybir.dt.float32
BF16 = mybir.dt.bfloat16

B, N, D = 2, 2048, 1024
H, DH = 16, 64
NCORES = 8
HL = H // NCORES          # local heads per core = 2
CL = HL * DH              # local head channels = 128
LN_EPS = 1e-5
SCALE = DH ** -0.5
KT = D // 128             # 8 d-tiles
JC = N // 128             # 16 j-chunks
IT = N // 512             # 4 i-tiles of 512

_CACHED_NC = None


def build_nc() -> bass.Bass:
    nc = bacc.Bacc(None)
    xt_d = nc.declare_dram_parameter("xt", [B, D, N], BF16, isOutput=False)
    al_d = nc.declare_dram_parameter("alibi", [HL, N, N], BF16, isOutput=False)
    wqkv_d = nc.declare_dram_parameter("wqkv", [D, 3 * CL], BF16, isOutput=False)
    qkvb_d = nc.declare_dram_parameter("qkvb", [3 * CL], F32, isOutput=False)
    nwsum_d = nc.declare_dram_parameter("nwsum", [3 * CL], F32, isOutput=False)
    wout_d = nc.declare_dram_parameter("wout", [CL, D], BF16, isOutput=False)
    ones_d = nc.declare_dram_parameter("ones", [128, 1], BF16, isOutput=False)
    out_d = nc.declare_dram_parameter("out", [B, D, N], BF16, isOutput=True)

    with tile.TileContext(nc) as tc, ExitStack() as ctx:
        ep = lambda **kw: ctx.enter_context(tc.tile_pool(**kw))
        cpool = ep(name="const", bufs=1)
        xt_pool = ep(name="xt", bufs=11)
        sq_pool = ep(name="sq", bufs=4)
        sm_pool = ep(name="small", bufs=2)
        tmp_pool = ep(name="tmp", bufs=4)
        qk_pool = ep(name="qk", bufs=1)      # per-batch tiles, all resident
        vt_pool = ep(name="vt", bufs=2)
        vn_pool = ep(name="vn", bufs=1)      # 4 resident tiles (b x head)
        al_pool = ep(name="al", bufs=8)
        at_pool = ep(name="at", bufs=8)
        ao_pool = ep(name="aos", bufs=1)
        ob_pool = ep(name="ob", bufs=4)
        bc_pool = ep(name="bc", bufs=4)
        rrbc_pool = ep(name="rrbc", bufs=3)
        aor_pool = ep(name="aor", bufs=3)
        dscr_pool = ep(name="dscr", bufs=2, space="DRAM")
        big_psum = ep(name="ps_big", bufs=3, space="PSUM")
        ao_psum = ep(name="ps_ao", bufs=1, space="PSUM")

        # ---- constants ----
        ident = cpool.tile([128, 128], BF16, name="ident")
        make_identity(nc, ident)
        zero_sb = cpool.tile([128, 1], F32, name="zero_sb")
        nc.vector.memset(zero_sb, 0.0)
        nc.const_aps.aps[(F32, 0.0)] = zero_sb[:, 0:1]
        eps_sb = cpool.tile([128, 1], F32, name="eps_sb")
        nc.vector.memset(eps_sb, LN_EPS)
        ones_sb = cpool.tile([128, 1], BF16, name="ones_sb")
        nc.sync.dma_start(out=ones_sb, in_=ones_d[:, :])
        # first batch's x tiles before the big weight loads: the opening
        # stats matmuls only need ones_sb + xt[0]
        xts0 = []
        for kt in range(KT):
            xt_t = xt_pool.tile([128, N], BF16, name=f"xt_0_{kt}", tag="xt")
            nc.sync.dma_start(out=xt_t, in_=xt_d[0, kt * 128:(kt + 1) * 128, :])
            xts0.append(xt_t)
        wqkv_sb = cpool.tile([128, KT, 3 * CL], BF16, name="wqkv_sb")
        nc.sync.dma_start(out=wqkv_sb, in_=wqkv_d.rearrange("(t p) c -> p t c", p=128))
        qkvb_sb = cpool.tile([128, 3], F32, name="qkvb_sb")
        nc.sync.dma_start(out=qkvb_sb, in_=qkvb_d.rearrange("(c p) -> p c", p=128))
        nwsum_sb = cpool.tile([128, 3], F32, name="nwsum_sb")
        nc.sync.dma_start(out=nwsum_sb, in_=nwsum_d.rearrange("(c p) -> p c", p=128))
        wout_sb = cpool.tile([128, D], BF16, name="wout_sb")
        nc.sync.dma_start(out=wout_sb, in_=wout_d[:, :])

        qTs, kTs, vns, aos = [], [], [], []
        for b in range(B):
            # ---- load xT (bf16) ----
            if b == 0:
                xts = xts0
            else:
                xts = []
                for kt in range(KT):
                    xt_t = xt_pool.tile([128, N], BF16, name=f"xt_{b}_{kt}", tag="xt")
                    nc.sync.dma_start(out=xt_t, in_=xt_d[b, kt * 128:(kt + 1) * 128, :])
                    xts.append(xt_t)

            # ---- LN stats (sum, sumsq) via matmul-with-ones ----

            scr = dscr_pool.tile([2, N], F32, name=f"scr_{b}", tag="scr")
            st = sm_pool.tile([128, 112], F32, name=f"st_{b}", tag="st128")
            for ihalf in range(2):
                isl = slice(ihalf * 1024, (ihalf + 1) * 1024)
                rows = sm_pool.tile([1, N], F32, name=f"rows_{b}_{ihalf}", tag="rows", bufs=1)
                sum_ps = big_psum.tile([1, 1024], F32, name=f"sum_{b}_{ihalf}", tag="big")
                sq_ps = big_psum.tile([33, 1024], F32, name=f"ssq_{b}_{ihalf}", tag="big")
                for kt in range(KT):
                    xsq = sq_pool.tile([128, 1024], BF16, name=f"xsq_{b}_{ihalf}_{kt}", tag="sq")
                    nc.vector.tensor_mul(xsq, xts[kt][:, isl], xts[kt][:, isl])
                    for it2 in range(2):
                        s2 = slice(it2 * 512, (it2 + 1) * 512)
                        i2 = slice(ihalf * 1024 + it2 * 512, ihalf * 1024 + (it2 + 1) * 512)
                        nc.tensor.matmul(
                            sum_ps[0:1, s2], ones_sb, xts[kt][:, i2],
                            start=(kt == 0), stop=(kt == KT - 1),
                        )
                        nc.tensor.matmul(
                            sq_ps[32:33, s2], ones_sb, xsq[:, s2],
                            start=(kt == 0), stop=(kt == KT - 1),
                            tile_position=(0, 32),
                        )
                nc.vector.tensor_copy(rows[0:1, 0:1024], sum_ps)
                nc.vector.tensor_copy(rows[0:1, 1024:2048], sq_ps[32:33, :])
                nc.sync.dma_start(out=st[:, ihalf * 8:(ihalf + 1) * 8], in_=rows[0:1, 0:1024])
                nc.sync.dma_start(out=st[:, 16 + ihalf * 8:16 + (ihalf + 1) * 8], in_=rows[0:1, 1024:2048])
            # stat cols: 0:16 sum128, 16:32 sumsq128, 32:48 mean, 48:64 ex2,
            # 64:80 -var, 80:96 rstd, 96:112 mean*rstd
            mean, ex2 = st[:, 32:48], st[:, 48:64]
            nvar, rstd, mrs = st[:, 64:80], st[:, 80:96], st[:, 96:112]
            nc.vector.tensor_scalar_mul(mean, st[:, 0:16], 1.0 / D)
            nc.vector.tensor_scalar_mul(ex2, st[:, 16:32], 1.0 / D)
            nc.vector.tensor_mul(nvar, mean, mean)
            nc.vector.tensor_sub(nvar, nvar, ex2)  # mean^2 - E[x^2] = -var
            nc.scalar.activation(
                rstd, nvar, mybir.ActivationFunctionType.Sqrt,
                bias=eps_sb[:, 0:1], scale=-1.0,
            )  # sqrt(var + eps)
            nc.vector.reciprocal(rstd, rstd)
            nc.vector.tensor_mul(mrs, mean, rstd)
            for ihalf in range(2):
                c8 = slice(ihalf * 8, (ihalf + 1) * 8)
                isl = slice(ihalf * 1024, (ihalf + 1) * 1024)
                nc.sync.dma_start(out=scr[0:1, isl], in_=rstd[:, c8])
                nc.sync.dma_start(out=scr[1:2, isl], in_=mrs[:, c8])
            # broadcast rows (DRAM -> 128 partitions)
            bcs = []
            for ihalf in range(2):
                isl = slice(ihalf * 1024, (ihalf + 1) * 1024)
                rstd_bc = bc_pool.tile([128, 1024], F32, name=f"rsbc_{b}_{ihalf}", tag="bc")
                nc.sync.dma_start(out=rstd_bc, in_=scr[0:1, isl].partition_broadcast(128))
                mrs_bc = bc_pool.tile([128, 1024], F32, name=f"mrbc_{b}_{ihalf}", tag="bc")
                nc.sync.dma_start(out=mrs_bc, in_=scr[1:2, isl].partition_broadcast(128))
                bcs.append((rstd_bc, mrs_bc))

            # ---- QKV projection on raw x; LN affine applied at eviction ----
            qT = qk_pool.tile([128, N], BF16, name=f"qT_{b}", tag=f"qT{b}")
            kT = qk_pool.tile([128, N], BF16, name=f"kT_{b}", tag=f"kT{b}")
            vT = vt_pool.tile([128, N], BF16, name=f"vT_{b}", tag="vT")
            qTs.append(qT)
            kTs.append(kT)
            sb_dst = [qT, kT, vT]
            for cc in (1, 2, 0):
                for ihalf in range(2):
                    isl = slice(ihalf * 1024, (ihalf + 1) * 1024)
                    pt = big_psum.tile([128, 1024], F32, name=f"qp_{b}_{cc}_{ihalf}", tag="big")
                    for kt in range(KT):
                        lhs = wqkv_sb[:, kt, cc * 128:(cc + 1) * 128]
                        for it2 in range(2):
                            s2 = slice(it2 * 512, (it2 + 1) * 512)
                            i2 = slice(ihalf * 1024 + it2 * 512, ihalf * 1024 + (it2 + 1) * 512)
                            bi = nc.tensor.matmul(
                                pt[:, s2], lhs, xts[kt][:, i2],
                                start=(kt == 0), stop=(kt == KT - 1),
                            )
                            if it2 == 1:
                                bi.ins.ldweights = False
                    rstd_bc, mrs_bc = bcs[ihalf]
                    tmp = tmp_pool.tile([128, 1024], F32, name=f"tmp_{b}_{cc}_{ihalf}", tag="tmp")
                    nc.vector.tensor_mul(tmp, pt, rstd_bc)
                    nc.vector.scalar_tensor_tensor(
                        out=sb_dst[cc][:, isl], in0=mrs_bc,
                        scalar=nwsum_sb[:, cc:cc + 1], in1=tmp,
                        op0=mybir.AluOpType.mult, op1=mybir.AluOpType.add,
                    )
                    nc.vector.tensor_scalar_add(
                        sb_dst[cc][:, isl], sb_dst[cc][:, isl], qkvb_sb[:, cc:cc + 1]
                    )

            # ---- v natural (+ ones column) via PE transpose ----
            vb = []
            for hh in range(HL):
                vn = vn_pool.tile([128, JC, DH + 1], BF16, name=f"vn_{b}_{hh}", tag=f"vn{b}{hh}")
                nc.gpsimd.memset(vn[:, :, DH:DH + 1], 1.0)
                vb.append(vn)
            vns.append(vb)
            for jc in range(JC):
                trp = big_psum.tile([128, 128], BF16, name=f"tr_{b}_{jc}", tag="big")
                nc.tensor.transpose(trp, vT[:, jc * 128:(jc + 1) * 128], ident)
                for hh in range(HL):
                    nc.vector.tensor_copy(
                        vb[hh][:, jc, 0:DH], trp[:, hh * DH:(hh + 1) * DH]
                    )

            ao_sb = ao_pool.tile([128, N], BF16, name=f"ao_{b}", tag=f"ao{b}")
            aos.append(ao_sb)

        # ---- attention: batch outer; ONE psum accumulator per group so two
        #      score tiles are in flight (PE never waits on ScalarE exp) ----
        scr3 = dscr_pool.tile([16, 1024], F32, name="scr3", tag="scr3")
        scr4 = dscr_pool.tile([16, 1024], F32, name="scr4", tag="scr4")
        for b in range(B):
            for hh in range(HL):
                hsl = slice(hh * DH, (hh + 1) * DH)
                for ihalf in range(2):
                    isl = slice(ihalf * 1024, (ihalf + 1) * 1024)
                    aop = ao_psum.tile([DH + 1, 1024], F32, name=f"aop_{b}_{hh}_{ihalf}", tag="aop")
                    for jc in range(JC):
                        jsl = slice(jc * 128, (jc + 1) * 128)
                        al_t = al_pool.tile([128, 1024], BF16, name=f"al_{b}_{hh}_{ihalf}_{jc}", tag="al")
                        nc.sync.dma_start(out=al_t, in_=al_d[hh, jsl, isl])
                        sc = big_psum.tile([128, 1024], F32, name=f"sc_{b}_{hh}_{ihalf}_{jc}", tag="big")
                        for it2 in range(2):
                            s2 = slice(it2 * 512, (it2 + 1) * 512)
                            i2 = slice(ihalf * 1024 + it2 * 512, ihalf * 1024 + (it2 + 1) * 512)
                            bi = nc.tensor.matmul(
                                sc[:, s2], kTs[b][hsl, jsl], qTs[b][hsl, i2],
                                start=True, stop=(it2 == 1),
                            )
                            if it2 == 1:
                                bi.ins.ldweights = False
                        nc.tensor.matmul(
                            sc[:, 0:512], ident, al_t[:, 0:512],
                            start=False, stop=True,
                        )
                        nc.vector.tensor_add(sc[:, 512:1024], sc[:, 512:1024], al_t[:, 512:1024])
                        at_t = at_pool.tile([128, 1024], BF16, name=f"at_{b}_{hh}_{ihalf}_{jc}", tag="at")
                        nc.scalar.activation(at_t, sc, mybir.ActivationFunctionType.Exp)
                        for it2 in range(2):
                            s2 = slice(it2 * 512, (it2 + 1) * 512)
                            bi = nc.tensor.matmul(
                                aop[:, s2], vns[b][hh][:, jc, :], at_t[:, s2],
                                start=(jc == 0), stop=(jc == JC - 1),
                            )
                            if it2 == 1:
                                bi.ins.ldweights = False
                    # evict raw attn output immediately (frees PSUM); the
                    # reciprocal runs at [128,8] via DMA reshapes and the
                    # normalize happens off the critical path
                    r = (hh * 2 + ihalf) * 2 + b
                    ao_raw = aor_pool.tile([DH + 1, 1024], F32, name=f"aor_{r}", tag="aor")
                    nc.vector.tensor_copy(ao_raw, aop)
                    nc.sync.dma_start(out=scr3[r:r + 1, :], in_=ao_raw[DH:DH + 1, :])
                    r128 = sm_pool.tile([128, 8], F32, name=f"r128_{r}", tag="r128", bufs=3)
                    nc.sync.dma_start(out=r128, in_=scr3[r:r + 1, :])
                    nc.vector.reciprocal(r128, r128)
                    nc.sync.dma_start(out=scr4[r:r + 1, :], in_=r128)
                    rr_bc = rrbc_pool.tile([DH, 1024], F32, name=f"rrbc_{r}", tag="rrbc")
                    nc.sync.dma_start(
                        out=rr_bc, in_=scr4[r:r + 1, :].partition_broadcast(DH)
                    )
                    nc.vector.tensor_mul(aos[b][hsl, isl], ao_raw[0:DH, :], rr_bc)

        # ---- out projection (partial, transposed, bf16) ----
        for ihalf in range(2):
            for b in range(B):
                for ec in range(8):
                    lhs = wout_sb[:, ec * 128:(ec + 1) * 128]
                    isl = slice(ihalf * 1024, (ihalf + 1) * 1024)
                    opp = big_psum.tile([128, 1024], F32, name=f"op_{b}_{ec}_{ihalf}", tag="big")
                    for it2 in range(2):
                        s2 = slice(it2 * 512, (it2 + 1) * 512)
                        i2 = slice(ihalf * 1024 + it2 * 512, ihalf * 1024 + (it2 + 1) * 512)
                        bi = nc.tensor.matmul(opp[:, s2], lhs, aos[b][:, i2], start=True, stop=True)
                        if it2 == 1:
                            bi.ins.ldweights = False
                    ob = ob_pool.tile([128, 1024], BF16, name=f"ob_{b}_{ec}_{ihalf}", tag="ob")
                    nc.vector.tensor_copy(ob, opp)
                    nc.sync.dma_start(out=out_d[b, ec * 128:(ec + 1) * 128, isl], in_=ob)
    nc.compile()
    return nc


def make_in_maps(x, alibi_bias, ln_gamma, ln_beta, w_qkv, w_out):
    """Host-side sharding / layout prep. Returns list of 8 per-core input dicts."""
    x = np.asarray(x, np.float32)
    alibi_bias = np.asarray(alibi_bias, np.float32)
    ln_gamma = np.asarray(ln_gamma, np.float32)
    ln_beta = np.asarray(ln_beta, np.float32)
    w_qkv = np.asarray(w_qkv, np.float32)
    w_out = np.asarray(w_out, np.float32)
    BF = ml_dtypes.bfloat16

    xt = np.ascontiguousarray(x.transpose(0, 2, 1)).astype(BF)  # [B, D, N]
    # fold ln_gamma into w_qkv rows; fold attention scale into the q columns
    w_eff = w_qkv * ln_gamma[:, None]
    qkvb_full = ln_beta @ w_qkv  # [3*H*DH]
    in_maps = []
    for c in range(NCORES):
        csl = slice(c * CL, (c + 1) * CL)
        wq = w_eff[:, 0:H * DH][:, csl] * SCALE
        wk = w_eff[:, H * DH:2 * H * DH][:, csl]
        wv = w_eff[:, 2 * H * DH:3 * H * DH][:, csl]
        wqkv_c = np.ascontiguousarray(np.concatenate([wq, wk, wv], axis=1)).astype(BF)
        nwsum_c = -wqkv_c.astype(np.float64).sum(axis=0).astype(np.float32)
        qb = qkvb_full.reshape(3, H * DH)[:, csl].copy()
        qb[0] *= SCALE
        qkvb_c = np.ascontiguousarray(qb.reshape(-1))
        al_c = np.ascontiguousarray(
            alibi_bias[0, c * HL:(c + 1) * HL].transpose(0, 2, 1)
        ).astype(BF)
        wout_c = np.ascontiguousarray(w_out[csl, :]).astype(BF)
        in_maps.append({
            "xt": xt,
            "alibi": al_c,
            "wqkv": wqkv_c,
            "qkvb": qkvb_c,
            "nwsum": nwsum_c,
            "wout": wout_c,
            "ones": np.ones((128, 1), BF),
        })
    return in_maps


def kernel(x, alibi_bias, mask, ln_gamma, ln_beta, w_qkv, w_out, _trace=False):
    global _CACHED_NC
    mask = np.asarray(mask)
    assert mask.all(), "kernel assumes an all-True mask"
    if _CACHED_NC is None:
        _CACHED_NC = build_nc()
    nc = _CACHED_NC
    in_maps = make_in_maps(x, alibi_bias, ln_gamma, ln_beta, w_qkv, w_out)
    res = run_bass_kernel_spmd(nc, in_maps, core_ids=list(range(NCORES)), trace=_trace)
    out_t = np.zeros((B, D, N), np.float32)
    for c in range(NCORES):
        out_t += res.results[c]["out"].astype(np.float32)
    out = np.ascontiguousarray(out_t.transpose(0, 2, 1))
    if _trace:
        return out, res
    return out
